# revision 1
# baseline (speedup 1.0000x reference)
"""Differential attention (B=2, S=2048, HS=1024, H=16, KV=4, D=64) on 8 trn2 cores.

Sharding: core c = (b, g) with b = c // 4 (data parallel on batch) and
g = c % 4 (tensor parallel over the 4 KV head groups; each core owns the
4 query heads of its group).  Each core computes its 4 heads' normed
attention output and a row-parallel partial of the output projection
(out_pt = (O_heads @ Wo_rows)^T); the host sums the 4 partials per batch.

Device pipeline per core (matmuls in fp32r ~= tf32, rel err ~3e-4 vs the
fp32 reference).  The emission is software-pipelined by q-tile —
proj(qt) -> rms(qt-1) -> attention(qt) -> wo(qt-1) — so the in-order
engine queues never head-of-line block ready work behind the RMS/output
latency chains:

  proj(qt): xT slice -> Q^T/K^T/V^T projections (PE).  RoPE needs
  swap32(q), which would be a cross-partition move; instead the host sends
  rotated+signed weight copies (wqr/wkr) so rot(Q)^T comes out of a second
  matmul and RoPE is 3 lane-aligned DVE ops: q*cos + qrot*sin.
  V^T is PE-transposed into [k,64] tiles with a 64-wide all-ones block
  appended ([V | ones]).

  attention(qt), per head: flash-style causal attention over k tiles,
  S^T[k,q] strips via two 64-contraction matmuls, P = exp(S/8) on ACT
  (no row-max: scores are O(5) so fp32 exp is safe; S/exp emitted
  STAGE=3 k-tiles ahead of the U matmuls to keep ACT saturated across
  the single-buffered psU boundary), causal diagonal wedge zeroed by
  gpsimd affine_select on P, U^T[128,q] += [V|ones].T @ P — the ones
  block replicates the softmax denominator onto partitions 64..127, so
  the epilogue is a lane-aligned reciprocal + one SBUF->SBUF partition
  shift + O = U1/r1 - lam*U2/r2 (lam folded into V2, subtract on gpsimd).

  rms(qt): ones-column matmul row-sums of O^2, rsqrt via ln/exp (same
  ACT table set as the softmax exp — no table thrash), gpsimd
  partition-broadcast, subln_w folded into Wo rows on the host.

  wo(qt): partial^T = Wo_rows.T @ O_norm^T -> DRAM.

PSUM: psS pairs [128,1024] double-buffered (4 banks) + psU [128,1024]
single (2) + aux (2).  DMAs are spread across the SP/ACT/Pool queues.
"""

import math
import sys

import numpy as np

try:
    import concourse.bass as bass  # noqa: F401
except ImportError:
    sys.path.insert(0, "/opt/trn_rl_repo")

import concourse.bass as bass
import concourse.tile as tile
from concourse import bacc, mybir
from concourse import bass_utils

f32 = mybir.dt.float32
f32r = mybir.dt.float32r
bf16 = mybir.dt.bfloat16
AF = mybir.ActivationFunctionType
ALU = mybir.AluOpType

B, S, HS = 2, 2048, 1024
H, KV, D = 16, 4, 64
NHL = 4            # query heads per core
NQT = 4            # q tiles of 512
QTW = 512
NKT = 16           # k tiles of 128
NHS = 8            # hs tiles of 128
NEG = -1e9
EPS = 1e-5

_prog_cache = {}


def _build_program(lam: float, repeat: int = 1, internal_io: bool = False):
    nc = bacc.Bacc("TRN2", target_bir_lowering=False, debug=False,
                   enable_asserts=False, num_devices=8)

    kin = "Internal" if internal_io else "ExternalInput"
    kout = "Internal" if internal_io else "ExternalOutput"
    xt = nc.dram_tensor("xt", [HS, S], f32r, kind=kin).ap()
    wq = nc.dram_tensor("wq", [HS, 512], f32r, kind=kin).ap()
    wqr = nc.dram_tensor("wqr", [HS, 512], f32r, kind=kin).ap()
    wk = nc.dram_tensor("wk", [HS, 128], f32r, kind=kin).ap()
    wkr = nc.dram_tensor("wkr", [HS, 128], f32r, kind=kin).ap()
    wv = nc.dram_tensor("wv", [HS, 64], f32r, kind=kin).ap()
    wo = nc.dram_tensor("wo", [256, HS], f32r, kind=kin).ap()
    cos_t = nc.dram_tensor("cos_t", [128, S], f32, kind=kin).ap()
    sin_t = nc.dram_tensor("sin_t", [128, S], f32, kind=kin).ap()
    idf = nc.dram_tensor("idf", [64, 64], f32, kind=kin).ap()
    ones = nc.dram_tensor("ones", [128, 64], f32r, kind=kin).ap()
    out_pt = nc.dram_tensor("out_pt", [HS, S], f32, kind=kout).ap()
    if internal_io:
        din = nc.dram_tensor("din", [1, 64], f32, kind="ExternalInput").ap()
        dout = nc.dram_tensor("dout", [1, 64], f32, kind="ExternalOutput").ap()

    with tile.TileContext(nc) as tc:
        with tc.tile_pool(name="persist", bufs=1) as pp, \
             tc.tile_pool(name="loc", bufs=2) as loc, \
             tc.tile_pool(name="pwk", bufs=2) as pwk, \
             tc.tile_pool(name="patt", bufs=5) as pa, \
             tc.tile_pool(name="ep", bufs=2) as pe, \
             tc.tile_pool(name="rmsp", bufs=2) as prm, \
             tc.psum_pool(name="ps", bufs=2) as ps_:

            _dma_engines = [nc.sync, nc.scalar, nc.gpsimd]
            _dma_i = [0]

            def dma_rr(dst, src):
                eng = _dma_engines[_dma_i[0] % 3]
                _dma_i[0] += 1
                eng.dma_start(dst, src)

            # load order = first-use order: wq/wqr feed the very first matmuls
            wq_sb, wk_sb, wv_sb, wqr_sb, wkr_sb = [], [], [], [], []
            for hs in range(NHS):
                t_ = pp.tile([128, 512], f32r, name=f"wq{hs}", tag=f"wq{hs}")
                dma_rr(t_[:], wq[hs * 128:(hs + 1) * 128, :])
                wq_sb.append(t_)
            for hs in range(NHS):
                t_ = pp.tile([128, 512], f32r, name=f"wqr{hs}", tag=f"wqr{hs}")
                dma_rr(t_[:], wqr[hs * 128:(hs + 1) * 128, :])
                wqr_sb.append(t_)
            for hs in range(NHS):
                t_ = pp.tile([128, 128], f32r, name=f"wk{hs}", tag=f"wk{hs}")
                dma_rr(t_[:], wk[hs * 128:(hs + 1) * 128, :])
                wk_sb.append(t_)
                t_ = pp.tile([128, 128], f32r, name=f"wkr{hs}", tag=f"wkr{hs}")
                dma_rr(t_[:], wkr[hs * 128:(hs + 1) * 128, :])
                wkr_sb.append(t_)
                t_ = pp.tile([128, 64], f32r, name=f"wv{hs}", tag=f"wv{hs}")
                dma_rr(t_[:], wv[hs * 128:(hs + 1) * 128, :])
                wv_sb.append(t_)
            ones_sb = pp.tile([128, 64], f32r, name="ones", tag="ones")
            dma_rr(ones_sb[:], ones[:])
            idf_sb = pp.tile([64, 64], f32, name="idf", tag="idf")
            dma_rr(idf_sb[:], idf[:])
            wo_sb = []
            for t in range(2):
                w = pp.tile([128, HS], f32r, name=f"wo{t}", tag=f"wo{t}")
                dma_rr(w[:], wo[t * 128:(t + 1) * 128, :])
                wo_sb.append(w)
            k_sb = pp.tile([128, S], f32r, name="k", tag="k")
            va = [pp.tile([128, 128], f32r, name=f"va{kt}", tag=f"va{kt}")
                  for kt in range(NKT)]
            vb = [pp.tile([128, 128], f32r, name=f"vb{kt}", tag=f"vb{kt}")
                  for kt in range(NKT)]
            for kt in range(NKT):
                nc.vector.tensor_copy(va[kt][:, 64:128], ones_sb[:])
                nc.vector.tensor_copy(vb[kt][:, 64:128], ones_sb[:])
            eps_sb = pp.tile([1, 1], f32, name="eps", tag="eps")
            nc.vector.memset(eps_sb[:], EPS)

            def rope_block(ps, psr, dst, cosq, sinq):
                # dst = ps * cos + psr * sin  (rotation+sign baked into wqr/wkr)
                qc = pwk.tile([128, QTW], f32, name="qc", tag="qc")
                nc.vector.tensor_mul(qc[:], ps[:], cosq[:])
                qs = pwk.tile([128, QTW], f32, name="qs", tag="qs")
                nc.vector.tensor_mul(qs[:], psr[:], sinq[:])
                nc.vector.tensor_add(dst, qc[:], qs[:])

            def emit_proj(qt, state):
                qlo, qhi = qt * QTW, (qt + 1) * QTW
                xt_sb = []
                for hs in range(NHS):
                    t_ = pwk.tile([128, QTW], f32r, name=f"xt{hs}", tag=f"xt{hs}",
                                  bufs=1)
                    nc.gpsimd.dma_start(t_[:], xt[hs * 128:(hs + 1) * 128,
                                                  qlo:qhi])
                    xt_sb.append(t_)
                cosq = loc.tile([128, QTW], f32, name="cosq", tag="cosq")
                nc.sync.dma_start(cosq[:], cos_t[:, qlo:qhi])
                sinq = loc.tile([128, QTW], f32, name="sinq", tag="sinq")
                nc.sync.dma_start(sinq[:], sin_t[:, qlo:qhi])
                qloc = [loc.tile([128, QTW], f32r, name=f"q{j}", tag=f"q{j}")
                        for j in range(NHL)]
                for j in range(NHL):
                    psq = ps_.tile([128, QTW], f32, name="psq", tag="aux")
                    for hs in range(NHS):
                        nc.tensor.matmul(
                            psq[:], wq_sb[hs][:, j * 128:(j + 1) * 128],
                            xt_sb[hs][:], start=(hs == 0), stop=(hs == NHS - 1))
                    psqr = ps_.tile([128, QTW], f32, name="psqr", tag="aux")
                    for hs in range(NHS):
                        nc.tensor.matmul(
                            psqr[:], wqr_sb[hs][:, j * 128:(j + 1) * 128],
                            xt_sb[hs][:], start=(hs == 0), stop=(hs == NHS - 1))
                    rope_block(psq, psqr, qloc[j][:], cosq, sinq)
                psk = ps_.tile([128, QTW], f32, name="psk", tag="aux")
                for hs in range(NHS):
                    nc.tensor.matmul(psk[:], wk_sb[hs][:], xt_sb[hs][:],
                                     start=(hs == 0), stop=(hs == NHS - 1))
                pskr = ps_.tile([128, QTW], f32, name="pskr", tag="aux")
                for hs in range(NHS):
                    nc.tensor.matmul(pskr[:], wkr_sb[hs][:], xt_sb[hs][:],
                                     start=(hs == 0), stop=(hs == NHS - 1))
                rope_block(psk, pskr, k_sb[:, qlo:qhi], cosq, sinq)
                psv = ps_.tile([64, QTW], f32, name="psv", tag="aux")
                for hs in range(NHS):
                    nc.tensor.matmul(psv[:], wv_sb[hs][:], xt_sb[hs][:],
                                     start=(hs == 0), stop=(hs == NHS - 1))
                vtq = loc.tile([64, QTW], f32, name="vtq", tag="vtq")
                nc.vector.tensor_copy(vtq[:], psv[:])
                for kk in range(4):
                    kt = 4 * qt + kk
                    psvt = ps_.tile([128, 64], f32, name="psvt", tag="aux")
                    nc.tensor.transpose(psvt[:], vtq[:, kk * 128:(kk + 1) * 128],
                                        idf_sb[:])
                    nc.vector.tensor_copy(va[kt][:, 0:64], psvt[:])
                    nc.vector.tensor_scalar_mul(vb[kt][:, 0:64], psvt[:], lam)
                state[qt] = qloc

            def emit_attention(qt, state, fuse_ssq=False):
                qloc = state[qt]
                if fuse_ssq:
                    ssqr = prm.tile([1, 4 * QTW], f32, name="ssqr", tag="rmsrow")
                    state[(qt, "ssqr")] = ssqr
                opair = [loc.tile([128, QTW], f32, name=f"op{t}", tag=f"op{t}")
                         for t in range(2)]
                onq = [loc.tile([128, QTW], f32r, name=f"on{t}", tag=f"on{t}")
                       for t in range(2)]
                state[(qt, "op")] = opair
                state[(qt, "on")] = onq
                for j in range(NHL):
                    half, pt = (j % 2) * 64, j // 2
                    last_kt = 4 * qt + 3
                    psu = ps_.tile([128, 2 * QTW], f32, name="psu", tag="psU",
                                   bufs=1)
                    p12s = {}

                    def emit_s_exp(kt):
                        jd = kt - 4 * qt
                        q0 = 128 * jd if jd >= 0 else 0
                        pss = ps_.tile([128, 2 * QTW], f32, name="pss", tag="psS")
                        nc.tensor.matmul(
                            pss[:, q0:QTW],
                            k_sb[0:64, kt * 128:(kt + 1) * 128],
                            qloc[j][0:64, q0:QTW],
                            start=True, stop=True, skip_group_check=True)
                        nc.tensor.matmul(
                            pss[:, QTW + q0:2 * QTW],
                            k_sb[64:128, kt * 128:(kt + 1) * 128],
                            qloc[j][64:128, q0:QTW],
                            start=True, stop=True, skip_group_check=True)
                        p12 = pa.tile([128, 2 * QTW], f32r, name="p12", tag="p12")
                        nc.scalar.activation(p12[:, q0:2 * QTW], pss[:, q0:2 * QTW],
                                             AF.Exp, scale=0.125)
                        if jd >= 0:
                            for off in (q0, QTW + q0):
                                nc.gpsimd.affine_select(
                                    p12[:, off:off + 128], p12[:, off:off + 128],
                                    pattern=[[1, 128]], compare_op=ALU.is_ge,
                                    fill=0.0, base=0, channel_multiplier=-1)
                        p12s[kt] = p12

                    STAGE = 5
                    for kt in range(min(STAGE, last_kt + 1)):
                        emit_s_exp(kt)
                    for kt in range(last_kt + 1):
                        if kt + STAGE <= last_kt:
                            emit_s_exp(kt + STAGE)
                        jd = kt - 4 * qt
                        q0 = 128 * jd if jd >= 0 else 0
                        p12 = p12s.pop(kt)
                        nc.tensor.matmul(
                            psu[:, q0:QTW], va[kt][:], p12[:, q0:QTW],
                            start=(kt == 0), stop=(kt == last_kt),
                            skip_group_check=True)
                        nc.tensor.matmul(
                            psu[:, QTW + q0:2 * QTW], vb[kt][:],
                            p12[:, QTW + q0:2 * QTW],
                            start=(kt == 0), stop=(kt == last_kt),
                            skip_group_check=True)
                    # epilogue: O^T = U1/r1 - lam*U2/r2  (no PE ops here)
                    wri = pe.tile([128, 2 * QTW], f32, name="wri", tag="wri")
                    nc.vector.reciprocal(wri[64:128, :], psu[64:128, :])
                    nc.sync.dma_start(wri[0:64, :], wri[64:128, :])
                    t1 = pe.tile([64, QTW], f32, name="t1", tag="t1")
                    nc.vector.tensor_mul(t1[:], psu[0:64, 0:QTW], wri[0:64, 0:QTW])
                    t2 = pe.tile([64, QTW], f32, name="t2", tag="t2")
                    nc.vector.tensor_mul(t2[:], psu[0:64, QTW:2 * QTW],
                                         wri[0:64, QTW:2 * QTW])
                    if j % 2 == 0:
                        nc.gpsimd.tensor_sub(opair[pt][0:64, :], t1[:], t2[:])
                    else:
                        otmp = pe.tile([64, QTW], f32, name="otmp", tag="otmp")
                        nc.gpsimd.tensor_sub(otmp[:], t1[:], t2[:])
                        nc.sync.dma_start(opair[pt][64:128, :], otmp[:])
                    if fuse_ssq:
                        osq = prm.tile([128, QTW], f32r, name="osq", tag="osq",
                                       bufs=1)
                        nc.vector.tensor_mul(osq[half:half + 64, :],
                                             opair[pt][half:half + 64, :],
                                             opair[pt][half:half + 64, :])
                        psss = ps_.tile([1, QTW], f32, name="psss", tag="aux")
                        nc.tensor.matmul(psss[:], ones_sb[half:half + 64, 0:1],
                                         osq[half:half + 64, :],
                                         start=True, stop=True)
                        nc.vector.tensor_copy(
                            ssqr[0:1, j * QTW:(j + 1) * QTW], psss[:])

            def emit_rms(qt, state):
                opair = state[(qt, "op")]
                onq = state[(qt, "on")]
                if (qt, "ssqr") in state:
                    ssqr = state[(qt, "ssqr")]
                else:
                    ssqr = prm.tile([1, 4 * QTW], f32, name="ssqr", tag="rmsrow")
                    for j in range(NHL):
                        half, pt = (j % 2) * 64, j // 2
                        osq = prm.tile([128, QTW], f32r, name="osq", tag="osq",
                                       bufs=1)
                        nc.vector.tensor_mul(osq[half:half + 64, :],
                                             opair[pt][half:half + 64, :],
                                             opair[pt][half:half + 64, :])
                        psss = ps_.tile([1, QTW], f32, name="psss", tag="aux")
                        nc.tensor.matmul(psss[:], ones_sb[half:half + 64, 0:1],
                                         osq[half:half + 64, :],
                                         start=True, stop=True)
                        nc.vector.tensor_copy(ssqr[0:1, j * QTW:(j + 1) * QTW],
                                              psss[:])
                lnq = prm.tile([1, 4 * QTW], f32, name="lnq", tag="rmsrow")
                nc.scalar.activation(lnq[:], ssqr[:], AF.Ln, scale=1.0 / 64.0,
                                     bias=eps_sb[0:1, 0:1])
                rmq = prm.tile([1, 4 * QTW], f32, name="rmq", tag="rmsrow")
                nc.scalar.activation(rmq[:], lnq[:], AF.Exp, scale=-0.5)
                for j in range(NHL):
                    half, pt = (j % 2) * 64, j // 2
                    rsb = prm.tile([128, QTW], f32, name="rsb", tag="rsb", bufs=1)
                    nc.gpsimd.partition_broadcast(
                        rsb[:], rmq[0:1, j * QTW:(j + 1) * QTW])
                    nc.vector.tensor_mul(onq[pt][half:half + 64, :],
                                         opair[pt][half:half + 64, :],
                                         rsb[half:half + 64, :])

            def emit_wo(qt, state):
                qlo, qhi = qt * QTW, (qt + 1) * QTW
                onq = state[(qt, "on")]
                for oc in range(8):
                    psw = ps_.tile([128, QTW], f32, name="psw", tag="aux")
                    nc.tensor.matmul(psw[:], wo_sb[0][:, oc * 128:(oc + 1) * 128],
                                     onq[0][:], start=True, stop=False)
                    nc.tensor.matmul(psw[:], wo_sb[1][:, oc * 128:(oc + 1) * 128],
                                     onq[1][:], start=False, stop=True)
                    ow = prm.tile([128, QTW], f32, name="ow", tag="ow")
                    if oc % 2 == 0:
                        nc.scalar.copy(ow[:], psw[:])
                    else:
                        nc.vector.tensor_copy(ow[:], psw[:])
                    dma_rr(out_pt[oc * 128:(oc + 1) * 128, qlo:qhi], ow[:])

            for rep in range(repeat):
                state = {}
                for qt in range(NQT):
                    emit_proj(qt, state)
                    if qt > 0:
                        emit_rms(qt - 1, state)
                    emit_attention(qt, state)
                    if qt > 0:
                        emit_wo(qt - 1, state)
                emit_rms(NQT - 1, state)
                emit_wo(NQT - 1, state)
    if internal_io:
        # tiny external I/O so the PJRT wrapper has something to move
        with tile.TileContext(nc) as tc2:
            with tc2.tile_pool(name="dio", bufs=1) as dp:
                dt_ = dp.tile([1, 64], f32, name="dt_")
                nc.sync.dma_start(dt_[:], din[:])
                nc.sync.dma_start(dout[:], dt_[:])
    nc.compile()
    return nc


def get_program(lam: float, repeat: int = 1, internal_io: bool = False):
    key = (round(float(lam), 9), repeat, internal_io)
    if key not in _prog_cache:
        _prog_cache[key] = _build_program(float(lam), repeat, internal_io)
    return _prog_cache[key]


def ml_bf16():
    import ml_dtypes
    return ml_dtypes.bfloat16


def _host_inputs(x, rope_cos, rope_sin, Wq, Wk, Wv, Wo, subln_w, lam):
    cos_t = np.ascontiguousarray(np.tile(rope_cos.T, (4, 1))).astype(np.float32)
    sin_t = np.ascontiguousarray(np.tile(rope_sin.T, (4, 1))).astype(np.float32)
    idf = np.eye(64, dtype=np.float32)
    ones = np.ones((128, 64), np.float32)
    sub4 = np.tile(subln_w.astype(np.float32), 4)[:, None]

    in_maps = []
    for c in range(8):
        b, g = c // 4, c % 4
        xtc = np.ascontiguousarray(x[b].T).astype(np.float32)
        cols = []
        for j in range(NHL):
            h = 4 * g + j
            cols.append(Wq[:, h * 64:(h + 1) * 64])
            cols.append(Wq[:, (H + h) * 64:(H + h + 1) * 64])
        wq_c = np.ascontiguousarray(np.concatenate(cols, axis=1)).astype(np.float32)
        wk_c = np.ascontiguousarray(np.concatenate(
            [Wk[:, g * 64:(g + 1) * 64], Wk[:, (KV + g) * 64:(KV + g + 1) * 64]],
            axis=1)).astype(np.float32)
        wq_r = _rot_weights(wq_c)
        wk_r = _rot_weights(wk_c)
        wv_c = np.ascontiguousarray(Wv[:, g * 64:(g + 1) * 64]).astype(np.float32)
        wo_c = np.ascontiguousarray(
            Wo[g * 256:(g + 1) * 256, :] * sub4).astype(np.float32)
        in_maps.append({
            "xt": xtc, "wq": wq_c, "wqr": wq_r, "wk": wk_c, "wkr": wk_r,
            "wv": wv_c, "wo": wo_c,
            "cos_t": cos_t, "sin_t": sin_t, "idf": idf, "ones": ones,
        })
    return in_maps


def _rot_weights(w):
    # columns grouped in 64-blocks of head dims: rot(q)[d<32] = -q[d+32],
    # rot(q)[d>=32] = +q[d-32]  -> column permutation with sign on the weights
    out = np.empty_like(w)
    nb = w.shape[1] // 64
    for b in range(nb):
        blk = w[:, b * 64:(b + 1) * 64]
        out[:, b * 64:b * 64 + 32] = -blk[:, 32:64]
        out[:, b * 64 + 32:(b + 1) * 64] = blk[:, 0:32]
    return np.ascontiguousarray(out)


def _compute_lam(lambda_q1, lambda_k1, lambda_q2, lambda_k2):
    li = 0.8 - 0.6 * math.exp(-0.3)
    l1 = np.exp(np.dot(lambda_q1.astype(np.float32), lambda_k1.astype(np.float32)))
    l2 = np.exp(np.dot(lambda_q2.astype(np.float32), lambda_k2.astype(np.float32)))
    return float(l1 - l2 + li)


def _numpy_reference(x, rope_cos, rope_sin, attention_mask, Wq, Wk, Wv, Wo,
                     lambda_q1, lambda_k1, lambda_q2, lambda_k2, subln_w):
    """Pure-numpy fallback, only used if the mask is not the expected causal one."""
    bsz, seq_len, _ = x.shape

    def rope(t):
        c = np.concatenate([rope_cos, rope_cos], axis=-1)[None, None]
        s = np.concatenate([rope_sin, rope_sin], axis=-1)[None, None]
        t1, t2 = np.split(t, 2, axis=-1)
        rot = np.concatenate([-t2, t1], axis=-1)
        return t * c + rot * s

    q = (x @ Wq).reshape(bsz, seq_len, 2 * H, D)
    q1 = np.transpose(q[:, :, :H], (0, 2, 1, 3))
    q2 = np.transpose(q[:, :, H:], (0, 2, 1, 3))
    k = (x @ Wk).reshape(bsz, seq_len, 2 * KV, D)
    k1 = np.transpose(k[:, :, :KV], (0, 2, 1, 3))
    k2 = np.transpose(k[:, :, KV:], (0, 2, 1, 3))
    v = np.transpose((x @ Wv).reshape(bsz, seq_len, KV, D), (0, 2, 1, 3))
    q1, q2, k1, k2 = rope(q1), rope(q2), rope(k1), rope(k2)
    gr = H // KV
    k1 = np.repeat(k1, gr, axis=1)
    k2 = np.repeat(k2, gr, axis=1)
    v = np.repeat(v, gr, axis=1)
    scale = 1.0 / math.sqrt(D)

    def smax(a):
        a = a - a.max(axis=-1, keepdims=True)
        e = np.exp(a)
        return e / e.sum(axis=-1, keepdims=True)

    a1 = smax(np.einsum("bhqd,bhkd->bhqk", q1, k1) * scale + attention_mask)
    a2 = smax(np.einsum("bhqd,bhkd->bhqk", q2, k2) * scale + attention_mask)
    lam = _compute_lam(lambda_q1, lambda_k1, lambda_q2, lambda_k2)
    attn = a1 - lam * a2
    out = np.einsum("bhqk,bhkd->bhqd", attn, v)
    inv = 1.0 / np.sqrt(np.mean(out * out, axis=-1, keepdims=True) + EPS)
    out = out * inv * subln_w
    out = np.transpose(out, (0, 2, 1, 3)).reshape(bsz, seq_len, HS)
    return (out @ Wo).astype(np.float32)


LAST_RESULT = None


def kernel(x, rope_cos, rope_sin, attention_mask, Wq, Wk, Wv, Wo,
           lambda_q1, lambda_k1, lambda_q2, lambda_k2, subln_w):
    global LAST_RESULT
    x = np.asarray(x, np.float32)
    kk, qq = np.arange(S)[:, None], np.arange(S)[None, :]
    causal = np.where(qq <= kk, 0.0, NEG).astype(np.float32)[None, None]
    am = np.asarray(attention_mask, np.float32)
    if am.shape != (1, 1, S, S) or not np.array_equal(am, causal):
        return _numpy_reference(x, rope_cos, rope_sin, am, Wq, Wk, Wv, Wo,
                                lambda_q1, lambda_k1, lambda_q2, lambda_k2,
                                subln_w)

    lam = _compute_lam(lambda_q1, lambda_k1, lambda_q2, lambda_k2)
    nc = get_program(lam)
    in_maps = _host_inputs(x, np.asarray(rope_cos, np.float32),
                           np.asarray(rope_sin, np.float32),
                           np.asarray(Wq, np.float32), np.asarray(Wk, np.float32),
                           np.asarray(Wv, np.float32), np.asarray(Wo, np.float32),
                           np.asarray(subln_w, np.float32), lam)
    res = bass_utils.run_bass_kernel_spmd(nc, in_maps, core_ids=list(range(8)))
    LAST_RESULT = res
    y = np.zeros((B, S, HS), np.float32)
    for c in range(8):
        y[c // 4] += res.results[c]["out_pt"].T
    return y



# revision 7
# speedup vs baseline: 1.1304x; 1.1304x over previous
"""Differential attention (B=2, S=2048, HS=1024, H=16, KV=4, D=64) on 8 trn2 cores.

Sharding: core c = (b, g) with b = c // 4 (data parallel on batch) and
g = c % 4 (tensor parallel over the 4 KV head groups; each core owns the
4 query heads of its group).  Each core computes its 4 heads' normed
attention output and a row-parallel partial of the output projection
(out_pt = (O_heads @ Wo_rows)^T); the host sums the 4 partials per batch.

All matmul operands are bf16 (PSUM accumulation stays fp32), which the
cost model runs at 1 cycle/row with no small-tile penalty and which
halves DMA traffic and enables the DVE 2x/4x perf modes on the
element-wise tail.  RoPE: instead of a second full-contraction projection
with rotated weights, rot(Q)^T is produced by a cheap 128-contraction
matmul against a signed permutation matrix (rot(q)[d<32] = -q[d+32],
rot(q)[d>=32] = +q[d-32], applied per 64-block): q_sb = copy(psq) on
Pool, psqr = perm^T @ q_sb on PE, then rope is q_sb*cos + psqr*sin on
DVE.  V^T is computed directly per 128-wide k tile (lhsT = x^T chunk,
rhs = Wv chunk) so no PE transposes or extra copies are needed.

attention(qt), per head: flash-style causal attention over k tiles,
S^T[k,q] strips via two 64-contraction matmuls, P = exp(S/8) on ACT
(no row-max: scores are O(5) so exp is safe; S/exp emitted STAGE k-tiles
ahead of the U matmuls), causal diagonal wedge zeroed by affine_select
(split across DVE and Pool), U^T[128,q] += [V|ones].T @ P — the ones
block replicates the softmax denominator onto partitions 64..127, so the
epilogue is a lane-aligned reciprocal + one SBUF->SBUF partition shift +
O = U1/r1 - lam*U2/r2 (lam folded into V2, subtract on gpsimd).

rms(qt): O^2 row-sums via four ones-column matmuls into partitions
{0,32,64,96} of one PSUM tile, a single strided-partition Sqrt on ACT +
reciprocal on DVE, gpsimd partition-broadcast, subln_w folded into Wo.

wo(qt): partial^T = Wo_rows.T @ O_norm^T -> DRAM (bf16, host upcasts).

The emission is software-pipelined by q-tile — proj(qt) -> rms(qt-1) ->
attention(qt) -> wo(qt-1) — so the in-order engine queues never
head-of-line block ready work behind the RMS/output latency chains.
"""

import math
import sys

import numpy as np

try:
    import concourse.bass as bass  # noqa: F401
except ImportError:
    sys.path.insert(0, "/opt/trn_rl_repo")

import concourse.bass as bass
import concourse.tile as tile
from concourse import bacc, mybir
from concourse import bass_utils

f32 = mybir.dt.float32
bf16 = mybir.dt.bfloat16
AF = mybir.ActivationFunctionType
ALU = mybir.AluOpType

B, S, HS = 2, 2048, 1024
H, KV, D = 16, 4, 64
NHL = 4            # query heads per core
NQT = 4            # q tiles of 512
QTW = 512
NKT = 16           # k tiles of 128
NHS = 8            # hs tiles of 128
NEG = -1e9
EPS = 1e-5

_prog_cache = {}


def _build_program(lam: float):
    nc = bacc.Bacc("TRN2", target_bir_lowering=False, debug=False,
                   enable_asserts=False, num_devices=8)

    xt = nc.dram_tensor("xt", [HS, S], bf16, kind="ExternalInput").ap()
    wq = nc.dram_tensor("wq", [HS, 512], bf16, kind="ExternalInput").ap()
    wk = nc.dram_tensor("wk", [HS, 128], bf16, kind="ExternalInput").ap()
    wv = nc.dram_tensor("wv", [HS, 64], bf16, kind="ExternalInput").ap()
    wo = nc.dram_tensor("wo", [256, HS], bf16, kind="ExternalInput").ap()
    perm = nc.dram_tensor("perm", [128, 128], bf16, kind="ExternalInput").ap()
    cos_t = nc.dram_tensor("cos_t", [128, S], bf16, kind="ExternalInput").ap()
    sin_t = nc.dram_tensor("sin_t", [128, S], bf16, kind="ExternalInput").ap()
    ones = nc.dram_tensor("ones", [128, 64], bf16, kind="ExternalInput").ap()
    trimask = nc.dram_tensor("trimask", [128, 128], bf16,
                             kind="ExternalInput").ap()
    out_pt = nc.dram_tensor("out_pt", [HS, S], bf16, kind="ExternalOutput").ap()

    with tile.TileContext(nc) as tc:
        with tc.tile_pool(name="persist", bufs=1) as pp, \
             tc.tile_pool(name="loc", bufs=2) as loc, \
             tc.tile_pool(name="pwk", bufs=2) as pwk, \
             tc.tile_pool(name="patt", bufs=5) as pa, \
             tc.tile_pool(name="ep", bufs=2) as pe, \
             tc.tile_pool(name="rmsp", bufs=2) as prm, \
             tc.psum_pool(name="ps", bufs=2) as ps_:

            _dma_engines = [nc.sync, nc.gpsimd]
            _dma_i = [0]

            def dma_rr(dst, src):
                eng = _dma_engines[_dma_i[0] % 2]
                _dma_i[0] += 1
                eng.dma_start(dst, src)

            # load order = first-use order: wq feeds the very first matmuls
            wq_sb, wk_sb, wv_sb = [], [], []
            for hs in range(NHS):
                t_ = pp.tile([128, 512], bf16, name=f"wq{hs}", tag=f"wq{hs}")
                dma_rr(t_[:], wq[hs * 128:(hs + 1) * 128, :])
                wq_sb.append(t_)
            perm_sb = pp.tile([128, 128], bf16, name="perm", tag="perm")
            dma_rr(perm_sb[:], perm[:])
            for hs in range(NHS):
                t_ = pp.tile([128, 128], bf16, name=f"wk{hs}", tag=f"wk{hs}")
                dma_rr(t_[:], wk[hs * 128:(hs + 1) * 128, :])
                wk_sb.append(t_)
                t_ = pp.tile([128, 64], bf16, name=f"wv{hs}", tag=f"wv{hs}")
                dma_rr(t_[:], wv[hs * 128:(hs + 1) * 128, :])
                wv_sb.append(t_)
            cos_sb = pp.tile([128, S], bf16, name="cos", tag="cos")
            dma_rr(cos_sb[:], cos_t[:])
            sin_sb = pp.tile([128, S], bf16, name="sin", tag="sin")
            dma_rr(sin_sb[:], sin_t[:])
            ones_sb = pp.tile([128, 64], bf16, name="ones", tag="ones")
            dma_rr(ones_sb[:], ones[:])
            tri_sb = pp.tile([128, 128], bf16, name="tri", tag="tri")
            dma_rr(tri_sb[:], trimask[:])
            wo_sb = []
            for t in range(2):
                w = pp.tile([128, HS], bf16, name=f"wo{t}", tag=f"wo{t}")
                dma_rr(w[:], wo[t * 128:(t + 1) * 128, :])
                wo_sb.append(w)
            k_sb = pp.tile([128, S], bf16, name="k", tag="k")
            va = [pp.tile([128, 128], bf16, name=f"va{kt}", tag=f"va{kt}")
                  for kt in range(NKT)]
            vb = [pp.tile([128, 128], bf16, name=f"vb{kt}", tag=f"vb{kt}")
                  for kt in range(NKT)]
            for kt in range(NKT):
                nc.vector.tensor_copy(va[kt][:, 64:128], ones_sb[:])
                nc.vector.tensor_copy(vb[kt][:, 64:128], ones_sb[:])
            eps_sb = pp.tile([128, 1], f32, name="eps", tag="eps")
            nc.vector.memset(eps_sb[:], EPS)

            def rope_block(ps, dst, qlo, qhi, dst_sb=None):
                # dst = q*cos + rot(q)*sin; rot via perm matmul on PE
                q_sb = dst_sb
                if q_sb is None:
                    q_sb = pwk.tile([128, QTW], bf16, name="qsb", tag="qsb")
                nc.gpsimd.tensor_copy(q_sb[:], ps[:])
                psr = ps_.tile([128, QTW], f32, name="psr", tag="aux")
                nc.tensor.matmul(psr[:], perm_sb[:], q_sb[:],
                                 start=True, stop=True)
                qc = pwk.tile([128, QTW], bf16, name="qc", tag="qc")
                nc.vector.tensor_mul(qc[:], q_sb[:], cos_sb[:, qlo:qhi])
                qs = pwk.tile([128, QTW], bf16, name="qs", tag="qs")
                nc.vector.tensor_mul(qs[:], psr[:], sin_sb[:, qlo:qhi])
                nc.vector.tensor_add(dst, qc[:], qs[:])

            def emit_proj(qt, state):
                qlo, qhi = qt * QTW, (qt + 1) * QTW
                xt_sb = []
                for hs in range(NHS):
                    t_ = pwk.tile([128, QTW], bf16, name=f"xt{hs}", tag=f"xt{hs}",
                                  bufs=1)
                    nc.gpsimd.dma_start(t_[:], xt[hs * 128:(hs + 1) * 128,
                                                  qlo:qhi])
                    xt_sb.append(t_)
                qloc = [loc.tile([128, QTW], bf16, name=f"q{j}", tag=f"q{j}")
                        for j in range(NHL)]
                for j in range(NHL):
                    psq = ps_.tile([128, QTW], f32, name="psq", tag="aux")
                    for hs in range(NHS):
                        nc.tensor.matmul(
                            psq[:], wq_sb[hs][:, j * 128:(j + 1) * 128],
                            xt_sb[hs][:], start=(hs == 0), stop=(hs == NHS - 1))
                    rope_block(psq, qloc[j][:], qlo, qhi)
                psk = ps_.tile([128, QTW], f32, name="psk", tag="aux")
                for hs in range(NHS):
                    nc.tensor.matmul(psk[:], wk_sb[hs][:], xt_sb[hs][:],
                                     start=(hs == 0), stop=(hs == NHS - 1))
                rope_block(psk, k_sb[:, qlo:qhi], qlo, qhi)
                for kk in range(4):
                    kt = 4 * qt + kk
                    psvt = ps_.tile([128, 64], f32, name="psvt", tag="aux")
                    for hs in range(NHS):
                        nc.tensor.matmul(
                            psvt[:], xt_sb[hs][:, kk * 128:(kk + 1) * 128],
                            wv_sb[hs][:], start=(hs == 0), stop=(hs == NHS - 1))
                    nc.vector.tensor_copy(va[kt][:, 0:64], psvt[:])
                    nc.vector.tensor_scalar_mul(vb[kt][:, 0:64], psvt[:], lam)
                state[qt] = qloc

            def emit_attention(qt, state):
                qloc = state[qt]
                opair = [loc.tile([128, QTW], bf16, name=f"op{t}", tag=f"op{t}")
                         for t in range(2)]
                onq = [loc.tile([128, QTW], bf16, name=f"on{t}", tag=f"on{t}")
                       for t in range(2)]
                state[(qt, "op")] = opair
                state[(qt, "on")] = onq
                for j in range(NHL):
                    half, pt = (j % 2) * 64, j // 2
                    last_kt = 4 * qt + 3
                    psu = ps_.tile([128, 2 * QTW], f32, name="psu", tag="psU",
                                   bufs=1)
                    p12s = {}

                    def emit_s_exp(kt):
                        jd = kt - 4 * qt
                        q0 = 128 * jd if jd >= 0 else 0
                        pss = ps_.tile([128, 2 * QTW], f32, name="pss", tag="psS")
                        nc.tensor.matmul(
                            pss[:, q0:QTW],
                            k_sb[0:64, kt * 128:(kt + 1) * 128],
                            qloc[j][0:64, q0:QTW],
                            start=True, stop=True, skip_group_check=True)
                        nc.tensor.matmul(
                            pss[:, QTW + q0:2 * QTW],
                            k_sb[64:128, kt * 128:(kt + 1) * 128],
                            qloc[j][64:128, q0:QTW],
                            start=True, stop=True, skip_group_check=True)
                        p12 = pa.tile([128, 2 * QTW], bf16, name="p12", tag="p12")
                        nc.scalar.activation(p12[:, q0:2 * QTW], pss[:, q0:2 * QTW],
                                             AF.Exp, scale=0.125)
                        if jd >= 0:
                            wap = p12[:].rearrange("p (b q) -> p b q",
                                                   b=2)[:, :, q0:q0 + 128]
                            msk = tri_sb[:].unsqueeze(1).broadcast_to(
                                [128, 2, 128])
                            nc.vector.tensor_mul(wap, wap, msk)
                        p12s[kt] = p12

                    STAGE = 5
                    for kt in range(min(STAGE, last_kt + 1)):
                        emit_s_exp(kt)
                    for kt in range(last_kt + 1):
                        if kt + STAGE <= last_kt:
                            emit_s_exp(kt + STAGE)
                        jd = kt - 4 * qt
                        q0 = 128 * jd if jd >= 0 else 0
                        p12 = p12s.pop(kt)
                        nc.tensor.matmul(
                            psu[:, q0:QTW], va[kt][:], p12[:, q0:QTW],
                            start=(kt == 0), stop=(kt == last_kt),
                            skip_group_check=True)
                        nc.tensor.matmul(
                            psu[:, QTW + q0:2 * QTW], vb[kt][:],
                            p12[:, QTW + q0:2 * QTW],
                            start=(kt == 0), stop=(kt == last_kt),
                            skip_group_check=True)
                    # epilogue: O^T = U1/r1 - lam*U2/r2  (no PE ops here)
                    wri = pe.tile([128, 2 * QTW], f32, name="wri", tag="wri")
                    nc.vector.reciprocal(wri[64:128, :], psu[64:128, :])
                    nc.sync.dma_start(wri[0:64, :], wri[64:128, :])
                    t1 = pe.tile([64, QTW], bf16, name="t1", tag="t1")
                    nc.vector.tensor_mul(t1[:], psu[0:64, 0:QTW], wri[0:64, 0:QTW])
                    t2 = pe.tile([64, QTW], bf16, name="t2", tag="t2")
                    nc.vector.tensor_mul(t2[:], psu[0:64, QTW:2 * QTW],
                                         wri[0:64, QTW:2 * QTW])
                    if j % 2 == 0:
                        nc.gpsimd.tensor_sub(opair[pt][0:64, :], t1[:], t2[:])
                    else:
                        otmp = pe.tile([64, QTW], bf16, name="otmp", tag="otmp")
                        nc.gpsimd.tensor_sub(otmp[:], t1[:], t2[:])
                        nc.sync.dma_start(opair[pt][64:128, :], otmp[:])

            def emit_rms(qt, state):
                opair = state[(qt, "op")]
                onq = state[(qt, "on")]
                sq4 = ps_.tile([128, QTW], f32, name="sq4", tag="aux")
                osqs = []
                for pt in range(2):
                    osq = prm.tile([128, QTW], bf16, name=f"osq{pt}",
                                   tag=f"osq{pt}")
                    nc.vector.tensor_mul(osq[:], opair[pt][:], opair[pt][:])
                    osqs.append(osq)
                for j in range(NHL):
                    half, pt = (j % 2) * 64, j // 2
                    nc.tensor.matmul(sq4[32 * j:32 * j + 1, :],
                                     ones_sb[half:half + 64, 0:1],
                                     osqs[pt][half:half + 64, :],
                                     start=True, stop=True,
                                     skip_group_check=True,
                                     tile_position=(half, 32 * j))
                # rms factor = 1/sqrt(ssq/64 + eps) on 4 strided partitions
                sqr = prm.tile([128, QTW], f32, name="sqr", tag="sqr")
                nc.scalar.activation(sqr[0:128:32, :], sq4[0:128:32, :],
                                     AF.Sqrt, scale=1.0 / 64.0,
                                     bias=eps_sb[0:128:32, 0:1])
                rmq = prm.tile([128, QTW], bf16, name="rmq", tag="rmq")
                with nc.allow_low_precision(reason="rms factor in bf16"):
                    nc.vector.reciprocal(rmq[0:128:32, :], sqr[0:128:32, :])
                for pt in range(2):
                    rsb = prm.tile([128, QTW], bf16, name="rsb", tag="rsb",
                                   bufs=1)
                    nc.gpsimd.partition_broadcast(
                        rsb[0:64, :], rmq[64 * pt:64 * pt + 1, :])
                    nc.gpsimd.partition_broadcast(
                        rsb[64:128, :], rmq[64 * pt + 32:64 * pt + 33, :])
                    nc.vector.tensor_mul(onq[pt][:], opair[pt][:], rsb[:])

            def emit_wo(qt, state):
                qlo, qhi = qt * QTW, (qt + 1) * QTW
                onq = state[(qt, "on")]
                for oc in range(8):
                    psw = ps_.tile([128, QTW], f32, name="psw", tag="aux")
                    nc.tensor.matmul(psw[:], wo_sb[0][:, oc * 128:(oc + 1) * 128],
                                     onq[0][:], start=True, stop=False)
                    nc.tensor.matmul(psw[:], wo_sb[1][:, oc * 128:(oc + 1) * 128],
                                     onq[1][:], start=False, stop=True)
                    ow = prm.tile([128, QTW], bf16, name="ow", tag="ow")
                    if oc % 2 == 0:
                        nc.vector.tensor_copy(ow[:], psw[:])
                    else:
                        nc.gpsimd.tensor_copy(ow[:], psw[:])
                    nc.sync.dma_start(out_pt[oc * 128:(oc + 1) * 128, qlo:qhi],
                                      ow[:])

            state = {}
            for qt in range(NQT):
                emit_proj(qt, state)
                if qt > 0:
                    emit_rms(qt - 1, state)
                emit_attention(qt, state)
                if qt > 0:
                    emit_wo(qt - 1, state)
            emit_rms(NQT - 1, state)
            emit_wo(NQT - 1, state)
    nc.compile()
    return nc


def get_program(lam: float):
    key = round(float(lam), 9)
    if key not in _prog_cache:
        _prog_cache[key] = _build_program(float(lam))
    return _prog_cache[key]


def _bf16():
    import ml_dtypes
    return ml_dtypes.bfloat16


def _perm_mat():
    # psr = perm.T @ q : psr[p] = -q[p+32] for p%64<32, +q[p-32] for p%64>=32
    p = np.zeros((128, 128), np.float32)
    for o in range(128):
        if o % 64 < 32:
            p[o + 32, o] = -1.0
        else:
            p[o - 32, o] = 1.0
    return p


def _host_inputs(x, rope_cos, rope_sin, Wq, Wk, Wv, Wo, subln_w, lam):
    bf = _bf16()
    cos_t = np.ascontiguousarray(np.tile(rope_cos.T, (4, 1))).astype(bf)
    sin_t = np.ascontiguousarray(np.tile(rope_sin.T, (4, 1))).astype(bf)
    perm = _perm_mat().astype(bf)
    ones = np.ones((128, 64), np.float32).astype(bf)
    tri = np.triu(np.ones((128, 128), np.float32)).astype(bf)
    sub4 = np.tile(subln_w.astype(np.float32), 4)[:, None]

    in_maps = []
    for c in range(8):
        b, g = c // 4, c % 4
        xtc = np.ascontiguousarray(x[b].T).astype(bf)
        cols = []
        for j in range(NHL):
            h = 4 * g + j
            cols.append(Wq[:, h * 64:(h + 1) * 64])
            cols.append(Wq[:, (H + h) * 64:(H + h + 1) * 64])
        wq_c = np.ascontiguousarray(np.concatenate(cols, axis=1)).astype(bf)
        wk_c = np.ascontiguousarray(np.concatenate(
            [Wk[:, g * 64:(g + 1) * 64], Wk[:, (KV + g) * 64:(KV + g + 1) * 64]],
            axis=1)).astype(bf)
        wv_c = np.ascontiguousarray(Wv[:, g * 64:(g + 1) * 64]).astype(bf)
        wo_c = np.ascontiguousarray(
            Wo[g * 256:(g + 1) * 256, :] * sub4).astype(bf)
        in_maps.append({
            "xt": xtc, "wq": wq_c, "wk": wk_c, "wv": wv_c, "wo": wo_c,
            "perm": perm, "cos_t": cos_t, "sin_t": sin_t, "ones": ones,
            "trimask": tri,
        })
    return in_maps


def _compute_lam(lambda_q1, lambda_k1, lambda_q2, lambda_k2):
    li = 0.8 - 0.6 * math.exp(-0.3)
    l1 = np.exp(np.dot(lambda_q1.astype(np.float32), lambda_k1.astype(np.float32)))
    l2 = np.exp(np.dot(lambda_q2.astype(np.float32), lambda_k2.astype(np.float32)))
    return float(l1 - l2 + li)


def _numpy_reference(x, rope_cos, rope_sin, attention_mask, Wq, Wk, Wv, Wo,
                     lambda_q1, lambda_k1, lambda_q2, lambda_k2, subln_w):
    """Pure-numpy fallback, only used if the mask is not the expected causal one."""
    bsz, seq_len, _ = x.shape

    def rope(t):
        c = np.concatenate([rope_cos, rope_cos], axis=-1)[None, None]
        s = np.concatenate([rope_sin, rope_sin], axis=-1)[None, None]
        t1, t2 = np.split(t, 2, axis=-1)
        rot = np.concatenate([-t2, t1], axis=-1)
        return t * c + rot * s

    q = (x @ Wq).reshape(bsz, seq_len, 2 * H, D)
    q1 = np.transpose(q[:, :, :H], (0, 2, 1, 3))
    q2 = np.transpose(q[:, :, H:], (0, 2, 1, 3))
    k = (x @ Wk).reshape(bsz, seq_len, 2 * KV, D)
    k1 = np.transpose(k[:, :, :KV], (0, 2, 1, 3))
    k2 = np.transpose(k[:, :, KV:], (0, 2, 1, 3))
    v = np.transpose((x @ Wv).reshape(bsz, seq_len, KV, D), (0, 2, 1, 3))
    q1, q2, k1, k2 = rope(q1), rope(q2), rope(k1), rope(k2)
    gr = H // KV
    k1 = np.repeat(k1, gr, axis=1)
    k2 = np.repeat(k2, gr, axis=1)
    v = np.repeat(v, gr, axis=1)
    scale = 1.0 / math.sqrt(D)

    def smax(a):
        a = a - a.max(axis=-1, keepdims=True)
        e = np.exp(a)
        return e / e.sum(axis=-1, keepdims=True)

    a1 = smax(np.einsum("bhqd,bhkd->bhqk", q1, k1) * scale + attention_mask)
    a2 = smax(np.einsum("bhqd,bhkd->bhqk", q2, k2) * scale + attention_mask)
    lam = _compute_lam(lambda_q1, lambda_k1, lambda_q2, lambda_k2)
    attn = a1 - lam * a2
    out = np.einsum("bhqk,bhkd->bhqd", attn, v)
    inv = 1.0 / np.sqrt(np.mean(out * out, axis=-1, keepdims=True) + EPS)
    out = out * inv * subln_w
    out = np.transpose(out, (0, 2, 1, 3)).reshape(bsz, seq_len, HS)
    return (out @ Wo).astype(np.float32)


LAST_RESULT = None


def kernel(x, rope_cos, rope_sin, attention_mask, Wq, Wk, Wv, Wo,
           lambda_q1, lambda_k1, lambda_q2, lambda_k2, subln_w):
    global LAST_RESULT
    x = np.asarray(x, np.float32)
    kk, qq = np.arange(S)[:, None], np.arange(S)[None, :]
    causal = np.where(qq <= kk, 0.0, NEG).astype(np.float32)[None, None]
    am = np.asarray(attention_mask, np.float32)
    if am.shape != (1, 1, S, S) or not np.array_equal(am, causal):
        return _numpy_reference(x, rope_cos, rope_sin, am, Wq, Wk, Wv, Wo,
                                lambda_q1, lambda_k1, lambda_q2, lambda_k2,
                                subln_w)

    lam = _compute_lam(lambda_q1, lambda_k1, lambda_q2, lambda_k2)
    nc = get_program(lam)
    in_maps = _host_inputs(x, np.asarray(rope_cos, np.float32),
                           np.asarray(rope_sin, np.float32),
                           np.asarray(Wq, np.float32), np.asarray(Wk, np.float32),
                           np.asarray(Wv, np.float32), np.asarray(Wo, np.float32),
                           np.asarray(subln_w, np.float32), lam)
    res = bass_utils.run_bass_kernel_spmd(nc, in_maps, core_ids=list(range(8)))
    LAST_RESULT = res
    y = np.zeros((B, S, HS), np.float32)
    for c in range(8):
        y[c // 4] += res.results[c]["out_pt"].T.astype(np.float32)
    return y


# revision 8
# speedup vs baseline: 1.1654x; 1.0309x over previous
"""Differential attention (B=2, S=2048, HS=1024, H=16, KV=4, D=64) on 8 trn2 cores.

Sharding: core c = (b, g) with b = c // 4 (data parallel on batch) and
g = c % 4 (tensor parallel over the 4 KV head groups; each core owns the
4 query heads of its group).  Each core computes its 4 heads' normed
attention output and a row-parallel partial of the output projection
(out_pt = (O_heads @ Wo_rows)^T); the host sums the 4 partials per batch.

All matmul operands are bf16 (PSUM accumulation stays fp32), which the
cost model runs at 1 cycle/row with no small-tile penalty and which
halves DMA traffic and enables the DVE 2x/4x perf modes on the
element-wise tail.  RoPE: instead of a second full-contraction projection
with rotated weights, rot(Q)^T is produced by a cheap 128-contraction
matmul against a signed permutation matrix (rot(q)[d<32] = -q[d+32],
rot(q)[d>=32] = +q[d-32], applied per 64-block): q_sb = copy(psq) on
Pool, psqr = perm^T @ q_sb on PE, then rope is q_sb*cos + psqr*sin on
DVE.  V^T is computed directly per 128-wide k tile (lhsT = x^T chunk,
rhs = Wv chunk) so no PE transposes or extra copies are needed.

attention(qt), per head: flash-style causal attention over k tiles,
S^T[k,q] strips via two 64-contraction matmuls, P = exp(S/8) on ACT
(no row-max: scores are O(5) so exp is safe; S/exp emitted STAGE k-tiles
ahead of the U matmuls), causal diagonal wedge zeroed by affine_select
(split across DVE and Pool), U^T[128,q] += [V|ones].T @ P — the ones
block replicates the softmax denominator onto partitions 64..127, so the
epilogue is a lane-aligned reciprocal + one SBUF->SBUF partition shift +
O = U1/r1 - lam*U2/r2 (lam folded into V2, subtract on gpsimd).

rms(qt): O^2 row-sums via four ones-column matmuls into partitions
{0,32,64,96} of one PSUM tile, a single strided-partition Sqrt on ACT +
reciprocal on DVE, gpsimd partition-broadcast, subln_w folded into Wo.

wo(qt): partial^T = Wo_rows.T @ O_norm^T -> DRAM (bf16, host upcasts).

The emission is software-pipelined by q-tile — proj(qt) -> rms(qt-1) ->
attention(qt) -> wo(qt-1) — so the in-order engine queues never
head-of-line block ready work behind the RMS/output latency chains.
"""

import math
import sys

import numpy as np

try:
    import concourse.bass as bass  # noqa: F401
except ImportError:
    sys.path.insert(0, "/opt/trn_rl_repo")

import concourse.bass as bass
import concourse.tile as tile
from concourse import bacc, mybir
from concourse import bass_utils

f32 = mybir.dt.float32
bf16 = mybir.dt.bfloat16
AF = mybir.ActivationFunctionType
ALU = mybir.AluOpType

B, S, HS = 2, 2048, 1024
H, KV, D = 16, 4, 64
NHL = 4            # query heads per core
NQT = 4            # q tiles of 512
QTW = 512
NKT = 16           # k tiles of 128
NHS = 8            # hs tiles of 128
NEG = -1e9
EPS = 1e-5

_prog_cache = {}


def _build_program(lam: float):
    nc = bacc.Bacc("TRN2", target_bir_lowering=False, debug=False,
                   enable_asserts=False, num_devices=8)

    xt = nc.dram_tensor("xt", [HS, S], bf16, kind="ExternalInput").ap()
    wq = nc.dram_tensor("wq", [HS, 512], bf16, kind="ExternalInput").ap()
    wk = nc.dram_tensor("wk", [HS, 128], bf16, kind="ExternalInput").ap()
    wv = nc.dram_tensor("wv", [HS, 64], bf16, kind="ExternalInput").ap()
    wo = nc.dram_tensor("wo", [256, HS], bf16, kind="ExternalInput").ap()
    perm = nc.dram_tensor("perm", [128, 128], bf16, kind="ExternalInput").ap()
    cos_t = nc.dram_tensor("cos_t", [128, S], bf16, kind="ExternalInput").ap()
    sin_t = nc.dram_tensor("sin_t", [128, S], bf16, kind="ExternalInput").ap()
    ones = nc.dram_tensor("ones", [128, 64], bf16, kind="ExternalInput").ap()
    trimask = nc.dram_tensor("trimask", [128, 128], bf16,
                             kind="ExternalInput").ap()
    out_pt = nc.dram_tensor("out_pt", [HS, S], bf16, kind="ExternalOutput").ap()

    with tile.TileContext(nc) as tc:
        with tc.tile_pool(name="persist", bufs=1) as pp, \
             tc.tile_pool(name="loc", bufs=2) as loc, \
             tc.tile_pool(name="pwk", bufs=2) as pwk, \
             tc.tile_pool(name="patt", bufs=5) as pa, \
             tc.tile_pool(name="ep", bufs=2) as pe, \
             tc.tile_pool(name="rmsp", bufs=2) as prm, \
             tc.psum_pool(name="ps", bufs=2) as ps_:

            _dma_engines = [nc.sync, nc.scalar]
            _dma_i = [0]

            def dma_rr(dst, src):
                eng = _dma_engines[_dma_i[0] % 2]
                _dma_i[0] += 1
                eng.dma_start(dst, src)

            # load order = first-use order: wq feeds the very first matmuls
            wq_sb, wk_sb, wv_sb = [], [], []
            for hs in range(NHS):
                t_ = pp.tile([128, 512], bf16, name=f"wq{hs}", tag=f"wq{hs}")
                dma_rr(t_[:], wq[hs * 128:(hs + 1) * 128, :])
                wq_sb.append(t_)
            perm_sb = pp.tile([128, 128], bf16, name="perm", tag="perm")
            dma_rr(perm_sb[:], perm[:])
            for hs in range(NHS):
                t_ = pp.tile([128, 128], bf16, name=f"wk{hs}", tag=f"wk{hs}")
                dma_rr(t_[:], wk[hs * 128:(hs + 1) * 128, :])
                wk_sb.append(t_)
                t_ = pp.tile([128, 64], bf16, name=f"wv{hs}", tag=f"wv{hs}")
                dma_rr(t_[:], wv[hs * 128:(hs + 1) * 128, :])
                wv_sb.append(t_)
            cos_sb = pp.tile([128, S], bf16, name="cos", tag="cos")
            dma_rr(cos_sb[:], cos_t[:])
            sin_sb = pp.tile([128, S], bf16, name="sin", tag="sin")
            dma_rr(sin_sb[:], sin_t[:])
            ones_sb = pp.tile([128, 64], bf16, name="ones", tag="ones")
            dma_rr(ones_sb[:], ones[:])
            tri_sb = pp.tile([128, 128], bf16, name="tri", tag="tri")
            dma_rr(tri_sb[:], trimask[:])
            wo_sb = []
            for t in range(2):
                w = pp.tile([128, HS], bf16, name=f"wo{t}", tag=f"wo{t}")
                dma_rr(w[:], wo[t * 128:(t + 1) * 128, :])
                wo_sb.append(w)
            k_sb = pp.tile([128, S], bf16, name="k", tag="k")
            va = [pp.tile([128, 128], bf16, name=f"va{kt}", tag=f"va{kt}")
                  for kt in range(NKT)]
            vb = [pp.tile([128, 128], bf16, name=f"vb{kt}", tag=f"vb{kt}")
                  for kt in range(NKT)]
            for kt in range(NKT):
                nc.vector.tensor_copy(va[kt][:, 64:128], ones_sb[:])
                nc.vector.tensor_copy(vb[kt][:, 64:128], ones_sb[:])
            eps_sb = pp.tile([128, 1], f32, name="eps", tag="eps")
            nc.vector.memset(eps_sb[:], EPS)

            def rope_block(ps, dst, qlo, qhi, dst_sb=None):
                # dst = q*cos + rot(q)*sin; rot via perm matmul on PE
                q_sb = dst_sb
                if q_sb is None:
                    q_sb = pwk.tile([128, QTW], bf16, name="qsb", tag="qsb")
                nc.gpsimd.tensor_copy(q_sb[:], ps[:])
                psr = ps_.tile([128, QTW], f32, name="psr", tag="aux")
                nc.tensor.matmul(psr[:], perm_sb[:], q_sb[:],
                                 start=True, stop=True)
                qc = pwk.tile([128, QTW], bf16, name="qc", tag="qc")
                nc.vector.tensor_mul(qc[:], q_sb[:], cos_sb[:, qlo:qhi])
                qs = pwk.tile([128, QTW], bf16, name="qs", tag="qs")
                nc.vector.tensor_mul(qs[:], psr[:], sin_sb[:, qlo:qhi])
                nc.vector.tensor_add(dst, qc[:], qs[:])

            def emit_proj(qt, state):
                qlo, qhi = qt * QTW, (qt + 1) * QTW
                xt_sb = []
                for hs in range(NHS):
                    t_ = pwk.tile([128, QTW], bf16, name=f"xt{hs}", tag=f"xt{hs}",
                                  bufs=1)
                    nc.scalar.dma_start(t_[:], xt[hs * 128:(hs + 1) * 128,
                                                  qlo:qhi])
                    xt_sb.append(t_)
                qloc = [loc.tile([128, QTW], bf16, name=f"q{j}", tag=f"q{j}")
                        for j in range(NHL)]
                for j in range(NHL):
                    psq = ps_.tile([128, QTW], f32, name="psq", tag="aux")
                    for hs in range(NHS):
                        nc.tensor.matmul(
                            psq[:], wq_sb[hs][:, j * 128:(j + 1) * 128],
                            xt_sb[hs][:], start=(hs == 0), stop=(hs == NHS - 1))
                    rope_block(psq, qloc[j][:], qlo, qhi)
                psk = ps_.tile([128, QTW], f32, name="psk", tag="aux")
                for hs in range(NHS):
                    nc.tensor.matmul(psk[:], wk_sb[hs][:], xt_sb[hs][:],
                                     start=(hs == 0), stop=(hs == NHS - 1))
                rope_block(psk, k_sb[:, qlo:qhi], qlo, qhi)
                for kk in range(4):
                    kt = 4 * qt + kk
                    psvt = ps_.tile([128, 64], f32, name="psvt", tag="aux")
                    for hs in range(NHS):
                        nc.tensor.matmul(
                            psvt[:], xt_sb[hs][:, kk * 128:(kk + 1) * 128],
                            wv_sb[hs][:], start=(hs == 0), stop=(hs == NHS - 1))
                    nc.vector.tensor_copy(va[kt][:, 0:64], psvt[:])
                    nc.vector.tensor_scalar_mul(vb[kt][:, 0:64], psvt[:], lam)
                state[qt] = qloc

            def emit_attention(qt, state):
                qloc = state[qt]
                opair = [loc.tile([128, QTW], bf16, name=f"op{t}", tag=f"op{t}")
                         for t in range(2)]
                onq = [loc.tile([128, QTW], bf16, name=f"on{t}", tag=f"on{t}")
                       for t in range(2)]
                state[(qt, "op")] = opair
                state[(qt, "on")] = onq
                for j in range(NHL):
                    half, pt = (j % 2) * 64, j // 2
                    last_kt = 4 * qt + 3
                    psu = ps_.tile([128, 2 * QTW], f32, name="psu", tag="psU",
                                   bufs=1)
                    p12s = {}

                    def emit_s_exp(kt):
                        jd = kt - 4 * qt
                        q0 = 128 * jd if jd >= 0 else 0
                        pss = ps_.tile([128, 2 * QTW], f32, name="pss", tag="psS")
                        nc.tensor.matmul(
                            pss[:, q0:QTW],
                            k_sb[0:64, kt * 128:(kt + 1) * 128],
                            qloc[j][0:64, q0:QTW],
                            start=True, stop=True, skip_group_check=True)
                        nc.tensor.matmul(
                            pss[:, QTW + q0:2 * QTW],
                            k_sb[64:128, kt * 128:(kt + 1) * 128],
                            qloc[j][64:128, q0:QTW],
                            start=True, stop=True, skip_group_check=True)
                        p12 = pa.tile([128, 2 * QTW], bf16, name="p12", tag="p12")
                        nc.scalar.activation(p12[:, q0:2 * QTW], pss[:, q0:2 * QTW],
                                             AF.Exp, scale=0.125)
                        if jd >= 0:
                            wap = p12[:].rearrange("p (b q) -> p b q",
                                                   b=2)[:, :, q0:q0 + 128]
                            msk = tri_sb[:].unsqueeze(1).broadcast_to(
                                [128, 2, 128])
                            nc.vector.tensor_mul(wap, wap, msk)
                        p12s[kt] = p12

                    STAGE = 5
                    for kt in range(min(STAGE, last_kt + 1)):
                        emit_s_exp(kt)
                    for kt in range(last_kt + 1):
                        if kt + STAGE <= last_kt:
                            emit_s_exp(kt + STAGE)
                        jd = kt - 4 * qt
                        q0 = 128 * jd if jd >= 0 else 0
                        p12 = p12s.pop(kt)
                        nc.tensor.matmul(
                            psu[:, q0:QTW], va[kt][:], p12[:, q0:QTW],
                            start=(kt == 0), stop=(kt == last_kt),
                            skip_group_check=True)
                        nc.tensor.matmul(
                            psu[:, QTW + q0:2 * QTW], vb[kt][:],
                            p12[:, QTW + q0:2 * QTW],
                            start=(kt == 0), stop=(kt == last_kt),
                            skip_group_check=True)
                    # epilogue: O^T = U1/r1 - lam*U2/r2  (no PE ops here)
                    wri = pe.tile([128, 2 * QTW], f32, name="wri", tag="wri")
                    nc.vector.reciprocal(wri[64:128, :], psu[64:128, :])
                    nc.sync.dma_start(wri[0:64, :], wri[64:128, :])
                    t1 = pe.tile([64, QTW], bf16, name="t1", tag="t1")
                    nc.vector.tensor_mul(t1[:], psu[0:64, 0:QTW], wri[0:64, 0:QTW])
                    t2 = pe.tile([64, QTW], bf16, name="t2", tag="t2")
                    nc.vector.tensor_mul(t2[:], psu[0:64, QTW:2 * QTW],
                                         wri[0:64, QTW:2 * QTW])
                    if j % 2 == 0:
                        nc.gpsimd.tensor_sub(opair[pt][0:64, :], t1[:], t2[:])
                    else:
                        otmp = pe.tile([64, QTW], bf16, name="otmp", tag="otmp")
                        nc.gpsimd.tensor_sub(otmp[:], t1[:], t2[:])
                        nc.sync.dma_start(opair[pt][64:128, :], otmp[:])

            def emit_rms(qt, state):
                opair = state[(qt, "op")]
                onq = state[(qt, "on")]
                sq4 = ps_.tile([128, QTW], f32, name="sq4", tag="aux")
                osqs = []
                for pt in range(2):
                    osq = prm.tile([128, QTW], bf16, name=f"osq{pt}",
                                   tag=f"osq{pt}")
                    nc.vector.tensor_mul(osq[:], opair[pt][:], opair[pt][:])
                    osqs.append(osq)
                for j in range(NHL):
                    half, pt = (j % 2) * 64, j // 2
                    nc.tensor.matmul(sq4[32 * j:32 * j + 1, :],
                                     ones_sb[half:half + 64, 0:1],
                                     osqs[pt][half:half + 64, :],
                                     start=True, stop=True,
                                     skip_group_check=True,
                                     tile_position=(half, 32 * j))
                # rms factor = exp(-0.5*ln(ssq/64+eps)) on 4 strided
                # partitions; Ln/Exp share the softmax act table (no reloads)
                sqr = prm.tile([128, QTW], f32, name="sqr", tag="sqr")
                nc.scalar.activation(sqr[0:128:32, :], sq4[0:128:32, :],
                                     AF.Ln, scale=1.0 / 64.0,
                                     bias=eps_sb[0:128:32, 0:1])
                rmq = prm.tile([128, QTW], bf16, name="rmq", tag="rmq")
                nc.scalar.activation(rmq[0:128:32, :], sqr[0:128:32, :],
                                     AF.Exp, scale=-0.5)
                for pt in range(2):
                    rsb = prm.tile([128, QTW], bf16, name="rsb", tag="rsb",
                                   bufs=1)
                    nc.gpsimd.partition_broadcast(
                        rsb[0:64, :], rmq[64 * pt:64 * pt + 1, :])
                    nc.gpsimd.partition_broadcast(
                        rsb[64:128, :], rmq[64 * pt + 32:64 * pt + 33, :])
                    nc.vector.tensor_mul(onq[pt][:], opair[pt][:], rsb[:])

            def emit_wo(qt, state):
                qlo, qhi = qt * QTW, (qt + 1) * QTW
                onq = state[(qt, "on")]
                for oc in range(8):
                    psw = ps_.tile([128, QTW], f32, name="psw", tag="aux")
                    nc.tensor.matmul(psw[:], wo_sb[0][:, oc * 128:(oc + 1) * 128],
                                     onq[0][:], start=True, stop=False)
                    nc.tensor.matmul(psw[:], wo_sb[1][:, oc * 128:(oc + 1) * 128],
                                     onq[1][:], start=False, stop=True)
                    ow = prm.tile([128, QTW], bf16, name="ow", tag="ow")
                    if oc % 2 == 0:
                        nc.vector.tensor_copy(ow[:], psw[:])
                    else:
                        nc.gpsimd.tensor_copy(ow[:], psw[:])
                    nc.sync.dma_start(out_pt[oc * 128:(oc + 1) * 128, qlo:qhi],
                                      ow[:])

            state = {}
            for qt in range(NQT):
                emit_proj(qt, state)
                if qt > 0:
                    emit_rms(qt - 1, state)
                emit_attention(qt, state)
                if qt > 0:
                    emit_wo(qt - 1, state)
            emit_rms(NQT - 1, state)
            emit_wo(NQT - 1, state)
    nc.compile()
    return nc


def get_program(lam: float):
    key = round(float(lam), 9)
    if key not in _prog_cache:
        _prog_cache[key] = _build_program(float(lam))
    return _prog_cache[key]


def _bf16():
    import ml_dtypes
    return ml_dtypes.bfloat16


def _perm_mat():
    # psr = perm.T @ q : psr[p] = -q[p+32] for p%64<32, +q[p-32] for p%64>=32
    p = np.zeros((128, 128), np.float32)
    for o in range(128):
        if o % 64 < 32:
            p[o + 32, o] = -1.0
        else:
            p[o - 32, o] = 1.0
    return p


def _host_inputs(x, rope_cos, rope_sin, Wq, Wk, Wv, Wo, subln_w, lam):
    bf = _bf16()
    cos_t = np.ascontiguousarray(np.tile(rope_cos.T, (4, 1))).astype(bf)
    sin_t = np.ascontiguousarray(np.tile(rope_sin.T, (4, 1))).astype(bf)
    perm = _perm_mat().astype(bf)
    ones = np.ones((128, 64), np.float32).astype(bf)
    tri = np.triu(np.ones((128, 128), np.float32)).astype(bf)
    sub4 = np.tile(subln_w.astype(np.float32), 4)[:, None]

    in_maps = []
    for c in range(8):
        b, g = c // 4, c % 4
        xtc = np.ascontiguousarray(x[b].T).astype(bf)
        cols = []
        for j in range(NHL):
            h = 4 * g + j
            cols.append(Wq[:, h * 64:(h + 1) * 64])
            cols.append(Wq[:, (H + h) * 64:(H + h + 1) * 64])
        wq_c = np.ascontiguousarray(np.concatenate(cols, axis=1)).astype(bf)
        wk_c = np.ascontiguousarray(np.concatenate(
            [Wk[:, g * 64:(g + 1) * 64], Wk[:, (KV + g) * 64:(KV + g + 1) * 64]],
            axis=1)).astype(bf)
        wv_c = np.ascontiguousarray(Wv[:, g * 64:(g + 1) * 64]).astype(bf)
        wo_c = np.ascontiguousarray(
            Wo[g * 256:(g + 1) * 256, :] * sub4).astype(bf)
        in_maps.append({
            "xt": xtc, "wq": wq_c, "wk": wk_c, "wv": wv_c, "wo": wo_c,
            "perm": perm, "cos_t": cos_t, "sin_t": sin_t, "ones": ones,
            "trimask": tri,
        })
    return in_maps


def _compute_lam(lambda_q1, lambda_k1, lambda_q2, lambda_k2):
    li = 0.8 - 0.6 * math.exp(-0.3)
    l1 = np.exp(np.dot(lambda_q1.astype(np.float32), lambda_k1.astype(np.float32)))
    l2 = np.exp(np.dot(lambda_q2.astype(np.float32), lambda_k2.astype(np.float32)))
    return float(l1 - l2 + li)


def _numpy_reference(x, rope_cos, rope_sin, attention_mask, Wq, Wk, Wv, Wo,
                     lambda_q1, lambda_k1, lambda_q2, lambda_k2, subln_w):
    """Pure-numpy fallback, only used if the mask is not the expected causal one."""
    bsz, seq_len, _ = x.shape

    def rope(t):
        c = np.concatenate([rope_cos, rope_cos], axis=-1)[None, None]
        s = np.concatenate([rope_sin, rope_sin], axis=-1)[None, None]
        t1, t2 = np.split(t, 2, axis=-1)
        rot = np.concatenate([-t2, t1], axis=-1)
        return t * c + rot * s

    q = (x @ Wq).reshape(bsz, seq_len, 2 * H, D)
    q1 = np.transpose(q[:, :, :H], (0, 2, 1, 3))
    q2 = np.transpose(q[:, :, H:], (0, 2, 1, 3))
    k = (x @ Wk).reshape(bsz, seq_len, 2 * KV, D)
    k1 = np.transpose(k[:, :, :KV], (0, 2, 1, 3))
    k2 = np.transpose(k[:, :, KV:], (0, 2, 1, 3))
    v = np.transpose((x @ Wv).reshape(bsz, seq_len, KV, D), (0, 2, 1, 3))
    q1, q2, k1, k2 = rope(q1), rope(q2), rope(k1), rope(k2)
    gr = H // KV
    k1 = np.repeat(k1, gr, axis=1)
    k2 = np.repeat(k2, gr, axis=1)
    v = np.repeat(v, gr, axis=1)
    scale = 1.0 / math.sqrt(D)

    def smax(a):
        a = a - a.max(axis=-1, keepdims=True)
        e = np.exp(a)
        return e / e.sum(axis=-1, keepdims=True)

    a1 = smax(np.einsum("bhqd,bhkd->bhqk", q1, k1) * scale + attention_mask)
    a2 = smax(np.einsum("bhqd,bhkd->bhqk", q2, k2) * scale + attention_mask)
    lam = _compute_lam(lambda_q1, lambda_k1, lambda_q2, lambda_k2)
    attn = a1 - lam * a2
    out = np.einsum("bhqk,bhkd->bhqd", attn, v)
    inv = 1.0 / np.sqrt(np.mean(out * out, axis=-1, keepdims=True) + EPS)
    out = out * inv * subln_w
    out = np.transpose(out, (0, 2, 1, 3)).reshape(bsz, seq_len, HS)
    return (out @ Wo).astype(np.float32)


LAST_RESULT = None


def kernel(x, rope_cos, rope_sin, attention_mask, Wq, Wk, Wv, Wo,
           lambda_q1, lambda_k1, lambda_q2, lambda_k2, subln_w):
    global LAST_RESULT
    x = np.asarray(x, np.float32)
    kk, qq = np.arange(S)[:, None], np.arange(S)[None, :]
    causal = np.where(qq <= kk, 0.0, NEG).astype(np.float32)[None, None]
    am = np.asarray(attention_mask, np.float32)
    if am.shape != (1, 1, S, S) or not np.array_equal(am, causal):
        return _numpy_reference(x, rope_cos, rope_sin, am, Wq, Wk, Wv, Wo,
                                lambda_q1, lambda_k1, lambda_q2, lambda_k2,
                                subln_w)

    lam = _compute_lam(lambda_q1, lambda_k1, lambda_q2, lambda_k2)
    nc = get_program(lam)
    in_maps = _host_inputs(x, np.asarray(rope_cos, np.float32),
                           np.asarray(rope_sin, np.float32),
                           np.asarray(Wq, np.float32), np.asarray(Wk, np.float32),
                           np.asarray(Wv, np.float32), np.asarray(Wo, np.float32),
                           np.asarray(subln_w, np.float32), lam)
    res = bass_utils.run_bass_kernel_spmd(nc, in_maps, core_ids=list(range(8)))
    LAST_RESULT = res
    y = np.zeros((B, S, HS), np.float32)
    for c in range(8):
        y[c // 4] += res.results[c]["out_pt"].T.astype(np.float32)
    return y


# revision 9
# speedup vs baseline: 1.1948x; 1.0252x over previous
"""Differential attention (B=2, S=2048, HS=1024, H=16, KV=4, D=64) on 8 trn2 cores.

Sharding: core c = (b, g) with b = c // 4 (data parallel on batch) and
g = c % 4 (tensor parallel over the 4 KV head groups; each core owns the
4 query heads of its group).  Each core computes its 4 heads' normed
attention output and a row-parallel partial of the output projection
(out_pt = (O_heads @ Wo_rows)^T); the host sums the 4 partials per batch.

All matmul operands are bf16 (PSUM accumulation stays fp32), which the
cost model runs at 1 cycle/row with no small-tile penalty and which
halves DMA traffic and enables the DVE 2x/4x perf modes on the
element-wise tail.  RoPE: instead of a second full-contraction projection
with rotated weights, rot(Q)^T is produced by a cheap 128-contraction
matmul against a signed permutation matrix (rot(q)[d<32] = -q[d+32],
rot(q)[d>=32] = +q[d-32], applied per 64-block): q_sb = copy(psq) on
Pool, psqr = perm^T @ q_sb on PE, then rope is q_sb*cos + psqr*sin on
DVE.  V^T is computed directly per 128-wide k tile (lhsT = x^T chunk,
rhs = Wv chunk) so no PE transposes or extra copies are needed.

attention(qt), per head: flash-style causal attention over k tiles,
S^T[k,q] strips via two 64-contraction matmuls, P = exp(S/8) on ACT
(no row-max: scores are O(5) so exp is safe; S/exp emitted STAGE k-tiles
ahead of the U matmuls), causal diagonal wedge zeroed by affine_select
(split across DVE and Pool), U^T[128,q] += [V|ones].T @ P — the ones
block replicates the softmax denominator onto partitions 64..127, so the
epilogue is a lane-aligned reciprocal + one SBUF->SBUF partition shift +
O = U1/r1 - lam*U2/r2 (lam folded into V2, subtract on gpsimd).

rms(qt): O^2 row-sums via four ones-column matmuls into partitions
{0,32,64,96} of one PSUM tile, a single strided-partition Sqrt on ACT +
reciprocal on DVE, gpsimd partition-broadcast, subln_w folded into Wo.

wo(qt): partial^T = Wo_rows.T @ O_norm^T -> DRAM (bf16, host upcasts).

The emission is software-pipelined by q-tile — proj(qt) -> rms(qt-1) ->
attention(qt) -> wo(qt-1) — so the in-order engine queues never
head-of-line block ready work behind the RMS/output latency chains.
"""

import math
import sys

import numpy as np

try:
    import concourse.bass as bass  # noqa: F401
except ImportError:
    sys.path.insert(0, "/opt/trn_rl_repo")

import concourse.bass as bass
import concourse.tile as tile
from concourse import bacc, mybir
from concourse import bass_utils

f32 = mybir.dt.float32
bf16 = mybir.dt.bfloat16
AF = mybir.ActivationFunctionType
ALU = mybir.AluOpType

B, S, HS = 2, 2048, 1024
H, KV, D = 16, 4, 64
NHL = 4            # query heads per core
NQT = 4            # q tiles of 512
QTW = 512
NKT = 16           # k tiles of 128
NHS = 8            # hs tiles of 128
NEG = -1e9
EPS = 1e-5

_prog_cache = {}


def _build_program(lam: float):
    nc = bacc.Bacc("TRN2", target_bir_lowering=False, debug=False,
                   enable_asserts=False, num_devices=8)

    xt = nc.dram_tensor("xt", [HS, S], bf16, kind="ExternalInput").ap()
    wq = nc.dram_tensor("wq", [HS, 512], bf16, kind="ExternalInput").ap()
    wk = nc.dram_tensor("wk", [HS, 128], bf16, kind="ExternalInput").ap()
    wv = nc.dram_tensor("wv", [HS, 64], bf16, kind="ExternalInput").ap()
    wo = nc.dram_tensor("wo", [256, HS], bf16, kind="ExternalInput").ap()
    perm = nc.dram_tensor("perm", [128, 128], bf16, kind="ExternalInput").ap()
    cos_t = nc.dram_tensor("cos_t", [128, S], bf16, kind="ExternalInput").ap()
    sin_t = nc.dram_tensor("sin_t", [128, S], bf16, kind="ExternalInput").ap()
    ones = nc.dram_tensor("ones", [128, 64], bf16, kind="ExternalInput").ap()
    trimask = nc.dram_tensor("trimask", [128, 128], bf16,
                             kind="ExternalInput").ap()
    out_pt = nc.dram_tensor("out_pt", [HS, S], bf16, kind="ExternalOutput").ap()

    with tile.TileContext(nc) as tc:
        with tc.tile_pool(name="persist", bufs=1) as pp, \
             tc.tile_pool(name="loc", bufs=2) as loc, \
             tc.tile_pool(name="pwk", bufs=2) as pwk, \
             tc.tile_pool(name="patt", bufs=5) as pa, \
             tc.tile_pool(name="ep", bufs=2) as pe, \
             tc.tile_pool(name="rmsp", bufs=2) as prm, \
             tc.psum_pool(name="ps", bufs=2) as ps_:

            _dma_engines = [nc.sync, nc.scalar]
            _dma_i = [0]

            def dma_rr(dst, src):
                eng = _dma_engines[_dma_i[0] % 2]
                _dma_i[0] += 1
                eng.dma_start(dst, src)

            # load order = first-use order: wq feeds the very first matmuls
            wq_sb, wk_sb, wv_sb = [], [], []
            for hs in range(NHS):
                t_ = pp.tile([128, 512], bf16, name=f"wq{hs}", tag=f"wq{hs}")
                dma_rr(t_[:], wq[hs * 128:(hs + 1) * 128, :])
                wq_sb.append(t_)
            perm_sb = pp.tile([128, 128], bf16, name="perm", tag="perm")
            dma_rr(perm_sb[:], perm[:])
            for hs in range(NHS):
                t_ = pp.tile([128, 128], bf16, name=f"wk{hs}", tag=f"wk{hs}")
                dma_rr(t_[:], wk[hs * 128:(hs + 1) * 128, :])
                wk_sb.append(t_)
                t_ = pp.tile([128, 64], bf16, name=f"wv{hs}", tag=f"wv{hs}")
                dma_rr(t_[:], wv[hs * 128:(hs + 1) * 128, :])
                wv_sb.append(t_)
            cos_sb = pp.tile([128, S], bf16, name="cos", tag="cos")
            dma_rr(cos_sb[:], cos_t[:])
            sin_sb = pp.tile([128, S], bf16, name="sin", tag="sin")
            dma_rr(sin_sb[:], sin_t[:])
            ones_sb = pp.tile([128, 64], bf16, name="ones", tag="ones")
            dma_rr(ones_sb[:], ones[:])
            tri_sb = pp.tile([128, 128], bf16, name="tri", tag="tri")
            dma_rr(tri_sb[:], trimask[:])
            wo_sb = []
            for t in range(2):
                w = pp.tile([128, HS], bf16, name=f"wo{t}", tag=f"wo{t}")
                dma_rr(w[:], wo[t * 128:(t + 1) * 128, :])
                wo_sb.append(w)
            k_sb = pp.tile([128, S], bf16, name="k", tag="k")
            va = [pp.tile([128, 128], bf16, name=f"va{kt}", tag=f"va{kt}")
                  for kt in range(NKT)]
            vb = [pp.tile([128, 128], bf16, name=f"vb{kt}", tag=f"vb{kt}")
                  for kt in range(NKT)]
            for kt in range(NKT):
                nc.vector.tensor_copy(va[kt][:, 64:128], ones_sb[:])
                nc.vector.tensor_copy(vb[kt][:, 64:128], ones_sb[:])
            eps_sb = pp.tile([128, 1], f32, name="eps", tag="eps")
            nc.vector.memset(eps_sb[:], EPS)
            # preload the act-func set that holds BOTH Exp and Ln so the
            # table-load pass never has to switch sets mid-stream
            from concourse.hw_specs import get_activation_tables
            _tables = list(get_activation_tables(nc.m.arch).items())
            _set_id = next(i for i, (_, fs) in enumerate(_tables)
                           if AF.Exp in fs and AF.Ln in fs)
            _ld = mybir.InstLoadActFuncSet(
                name=nc.get_next_instruction_name(),
                act_func_set_id=_set_id, ins=[], outs=[])
            nc.scalar.add_instruction(_ld)

            def rope_block(ps, dst, qlo, qhi, dst_sb=None):
                # dst = q*cos + rot(q)*sin; rot via perm matmul on PE
                q_sb = dst_sb
                if q_sb is None:
                    q_sb = pwk.tile([128, QTW], bf16, name="qsb", tag="qsb")
                nc.gpsimd.tensor_copy(q_sb[:], ps[:])
                psr = ps_.tile([128, QTW], f32, name="psr", tag="aux")
                nc.tensor.matmul(psr[:], perm_sb[:], q_sb[:],
                                 start=True, stop=True)
                qc = pwk.tile([128, QTW], bf16, name="qc", tag="qc")
                nc.vector.tensor_mul(qc[:], q_sb[:], cos_sb[:, qlo:qhi])
                qs = pwk.tile([128, QTW], bf16, name="qs", tag="qs")
                nc.vector.tensor_mul(qs[:], psr[:], sin_sb[:, qlo:qhi])
                nc.vector.tensor_add(dst, qc[:], qs[:])

            def emit_proj(qt, state):
                qlo, qhi = qt * QTW, (qt + 1) * QTW
                xt_sb = []
                for hs in range(NHS):
                    t_ = pwk.tile([128, QTW], bf16, name=f"xt{hs}", tag=f"xt{hs}",
                                  bufs=1)
                    nc.scalar.dma_start(t_[:], xt[hs * 128:(hs + 1) * 128,
                                                  qlo:qhi])
                    xt_sb.append(t_)
                qloc = [loc.tile([128, QTW], bf16, name=f"q{j}", tag=f"q{j}")
                        for j in range(NHL)]
                for j in range(NHL):
                    psq = ps_.tile([128, QTW], f32, name="psq", tag="aux")
                    for hs in range(NHS):
                        nc.tensor.matmul(
                            psq[:], wq_sb[hs][:, j * 128:(j + 1) * 128],
                            xt_sb[hs][:], start=(hs == 0), stop=(hs == NHS - 1))
                    rope_block(psq, qloc[j][:], qlo, qhi)
                psk = ps_.tile([128, QTW], f32, name="psk", tag="aux")
                for hs in range(NHS):
                    nc.tensor.matmul(psk[:], wk_sb[hs][:], xt_sb[hs][:],
                                     start=(hs == 0), stop=(hs == NHS - 1))
                rope_block(psk, k_sb[:, qlo:qhi], qlo, qhi)
                for kk in range(4):
                    kt = 4 * qt + kk
                    psvt = ps_.tile([128, 64], f32, name="psvt", tag="aux")
                    for hs in range(NHS):
                        nc.tensor.matmul(
                            psvt[:], xt_sb[hs][:, kk * 128:(kk + 1) * 128],
                            wv_sb[hs][:], start=(hs == 0), stop=(hs == NHS - 1))
                    nc.vector.tensor_copy(va[kt][:, 0:64], psvt[:])
                    nc.vector.tensor_scalar_mul(vb[kt][:, 0:64], psvt[:], lam)
                state[qt] = qloc

            def emit_attention(qt, state):
                qloc = state[qt]
                opair = [loc.tile([128, QTW], bf16, name=f"op{t}", tag=f"op{t}")
                         for t in range(2)]
                onq = [loc.tile([128, QTW], bf16, name=f"on{t}", tag=f"on{t}")
                       for t in range(2)]
                state[(qt, "op")] = opair
                state[(qt, "on")] = onq
                for j in range(NHL):
                    half, pt = (j % 2) * 64, j // 2
                    last_kt = 4 * qt + 3
                    psu = ps_.tile([128, 2 * QTW], f32, name="psu", tag="psU",
                                   bufs=1)
                    p12s = {}

                    def emit_s_exp(kt):
                        jd = kt - 4 * qt
                        q0 = 128 * jd if jd >= 0 else 0
                        pss = ps_.tile([128, 2 * QTW], f32, name="pss", tag="psS")
                        nc.tensor.matmul(
                            pss[:, q0:QTW],
                            k_sb[0:64, kt * 128:(kt + 1) * 128],
                            qloc[j][0:64, q0:QTW],
                            start=True, stop=True, skip_group_check=True)
                        nc.tensor.matmul(
                            pss[:, QTW + q0:2 * QTW],
                            k_sb[64:128, kt * 128:(kt + 1) * 128],
                            qloc[j][64:128, q0:QTW],
                            start=True, stop=True, skip_group_check=True)
                        p12 = pa.tile([128, 2 * QTW], bf16, name="p12", tag="p12")
                        nc.scalar.activation(p12[:, q0:2 * QTW], pss[:, q0:2 * QTW],
                                             AF.Exp, scale=0.125)
                        if jd >= 0:
                            wap = p12[:].rearrange("p (b q) -> p b q",
                                                   b=2)[:, :, q0:q0 + 128]
                            msk = tri_sb[:].unsqueeze(1).broadcast_to(
                                [128, 2, 128])
                            nc.vector.tensor_mul(wap, wap, msk)
                        p12s[kt] = p12

                    STAGE = 5
                    for kt in range(min(STAGE, last_kt + 1)):
                        emit_s_exp(kt)
                    for kt in range(last_kt + 1):
                        if kt + STAGE <= last_kt:
                            emit_s_exp(kt + STAGE)
                        jd = kt - 4 * qt
                        q0 = 128 * jd if jd >= 0 else 0
                        p12 = p12s.pop(kt)
                        nc.tensor.matmul(
                            psu[:, q0:QTW], va[kt][:], p12[:, q0:QTW],
                            start=(kt == 0), stop=(kt == last_kt),
                            skip_group_check=True)
                        nc.tensor.matmul(
                            psu[:, QTW + q0:2 * QTW], vb[kt][:],
                            p12[:, QTW + q0:2 * QTW],
                            start=(kt == 0), stop=(kt == last_kt),
                            skip_group_check=True)
                    # epilogue: O^T = U1/r1 - lam*U2/r2  (no PE ops here)
                    wri = pe.tile([128, 2 * QTW], f32, name="wri", tag="wri")
                    nc.vector.reciprocal(wri[64:128, :], psu[64:128, :])
                    nc.sync.dma_start(wri[0:64, :], wri[64:128, :])
                    t1 = pe.tile([64, QTW], bf16, name="t1", tag="t1")
                    nc.vector.tensor_mul(t1[:], psu[0:64, 0:QTW], wri[0:64, 0:QTW])
                    t2 = pe.tile([64, QTW], bf16, name="t2", tag="t2")
                    nc.vector.tensor_mul(t2[:], psu[0:64, QTW:2 * QTW],
                                         wri[0:64, QTW:2 * QTW])
                    if j % 2 == 0:
                        nc.gpsimd.tensor_sub(opair[pt][0:64, :], t1[:], t2[:])
                    else:
                        otmp = pe.tile([64, QTW], bf16, name="otmp", tag="otmp")
                        nc.gpsimd.tensor_sub(otmp[:], t1[:], t2[:])
                        nc.sync.dma_start(opair[pt][64:128, :], otmp[:])

            def emit_rms(qt, state):
                opair = state[(qt, "op")]
                onq = state[(qt, "on")]
                sq4 = ps_.tile([128, QTW], f32, name="sq4", tag="aux")
                osqs = []
                for pt in range(2):
                    osq = prm.tile([128, QTW], bf16, name=f"osq{pt}",
                                   tag=f"osq{pt}")
                    nc.vector.tensor_mul(osq[:], opair[pt][:], opair[pt][:])
                    osqs.append(osq)
                for j in range(NHL):
                    half, pt = (j % 2) * 64, j // 2
                    nc.tensor.matmul(sq4[32 * j:32 * j + 1, :],
                                     ones_sb[half:half + 64, 0:1],
                                     osqs[pt][half:half + 64, :],
                                     start=True, stop=True,
                                     skip_group_check=True,
                                     tile_position=(half, 32 * j))
                # rms factor = exp(-0.5*ln(ssq/64+eps)) on 4 strided
                # partitions; Ln/Exp share the softmax act table (no reloads)
                sqr = prm.tile([128, QTW], f32, name="sqr", tag="sqr")
                nc.scalar.activation(sqr[0:128:32, :], sq4[0:128:32, :],
                                     AF.Ln, scale=1.0 / 64.0,
                                     bias=eps_sb[0:128:32, 0:1])
                rmq = prm.tile([128, QTW], bf16, name="rmq", tag="rmq")
                nc.scalar.activation(rmq[0:128:32, :], sqr[0:128:32, :],
                                     AF.Exp, scale=-0.5)
                for pt in range(2):
                    rsb = prm.tile([128, QTW], bf16, name="rsb", tag="rsb",
                                   bufs=1)
                    nc.gpsimd.partition_broadcast(
                        rsb[0:64, :], rmq[64 * pt:64 * pt + 1, :])
                    nc.gpsimd.partition_broadcast(
                        rsb[64:128, :], rmq[64 * pt + 32:64 * pt + 33, :])
                    nc.vector.tensor_mul(onq[pt][:], opair[pt][:], rsb[:])

            def emit_wo(qt, state):
                qlo, qhi = qt * QTW, (qt + 1) * QTW
                onq = state[(qt, "on")]
                for oc in range(8):
                    psw = ps_.tile([128, QTW], f32, name="psw", tag="aux")
                    nc.tensor.matmul(psw[:], wo_sb[0][:, oc * 128:(oc + 1) * 128],
                                     onq[0][:], start=True, stop=False)
                    nc.tensor.matmul(psw[:], wo_sb[1][:, oc * 128:(oc + 1) * 128],
                                     onq[1][:], start=False, stop=True)
                    ow = prm.tile([128, QTW], bf16, name="ow", tag="ow")
                    if oc % 2 == 0:
                        nc.vector.tensor_copy(ow[:], psw[:])
                    else:
                        nc.gpsimd.tensor_copy(ow[:], psw[:])
                    nc.sync.dma_start(out_pt[oc * 128:(oc + 1) * 128, qlo:qhi],
                                      ow[:])

            state = {}
            for qt in range(NQT):
                emit_proj(qt, state)
                if qt > 0:
                    emit_rms(qt - 1, state)
                emit_attention(qt, state)
                if qt > 0:
                    emit_wo(qt - 1, state)
            emit_rms(NQT - 1, state)
            emit_wo(NQT - 1, state)
    nc.compile()
    return nc


def get_program(lam: float):
    key = round(float(lam), 9)
    if key not in _prog_cache:
        _prog_cache[key] = _build_program(float(lam))
    return _prog_cache[key]


def _bf16():
    import ml_dtypes
    return ml_dtypes.bfloat16


def _perm_mat():
    # psr = perm.T @ q : psr[p] = -q[p+32] for p%64<32, +q[p-32] for p%64>=32
    p = np.zeros((128, 128), np.float32)
    for o in range(128):
        if o % 64 < 32:
            p[o + 32, o] = -1.0
        else:
            p[o - 32, o] = 1.0
    return p


def _host_inputs(x, rope_cos, rope_sin, Wq, Wk, Wv, Wo, subln_w, lam):
    bf = _bf16()
    cos_t = np.ascontiguousarray(np.tile(rope_cos.T, (4, 1))).astype(bf)
    sin_t = np.ascontiguousarray(np.tile(rope_sin.T, (4, 1))).astype(bf)
    perm = _perm_mat().astype(bf)
    ones = np.ones((128, 64), np.float32).astype(bf)
    tri = np.triu(np.ones((128, 128), np.float32)).astype(bf)
    sub4 = np.tile(subln_w.astype(np.float32), 4)[:, None]

    in_maps = []
    for c in range(8):
        b, g = c // 4, c % 4
        xtc = np.ascontiguousarray(x[b].T).astype(bf)
        cols = []
        for j in range(NHL):
            h = 4 * g + j
            cols.append(Wq[:, h * 64:(h + 1) * 64])
            cols.append(Wq[:, (H + h) * 64:(H + h + 1) * 64])
        wq_c = np.ascontiguousarray(np.concatenate(cols, axis=1)).astype(bf)
        wk_c = np.ascontiguousarray(np.concatenate(
            [Wk[:, g * 64:(g + 1) * 64], Wk[:, (KV + g) * 64:(KV + g + 1) * 64]],
            axis=1)).astype(bf)
        wv_c = np.ascontiguousarray(Wv[:, g * 64:(g + 1) * 64]).astype(bf)
        wo_c = np.ascontiguousarray(
            Wo[g * 256:(g + 1) * 256, :] * sub4).astype(bf)
        in_maps.append({
            "xt": xtc, "wq": wq_c, "wk": wk_c, "wv": wv_c, "wo": wo_c,
            "perm": perm, "cos_t": cos_t, "sin_t": sin_t, "ones": ones,
            "trimask": tri,
        })
    return in_maps


def _compute_lam(lambda_q1, lambda_k1, lambda_q2, lambda_k2):
    li = 0.8 - 0.6 * math.exp(-0.3)
    l1 = np.exp(np.dot(lambda_q1.astype(np.float32), lambda_k1.astype(np.float32)))
    l2 = np.exp(np.dot(lambda_q2.astype(np.float32), lambda_k2.astype(np.float32)))
    return float(l1 - l2 + li)


def _numpy_reference(x, rope_cos, rope_sin, attention_mask, Wq, Wk, Wv, Wo,
                     lambda_q1, lambda_k1, lambda_q2, lambda_k2, subln_w):
    """Pure-numpy fallback, only used if the mask is not the expected causal one."""
    bsz, seq_len, _ = x.shape

    def rope(t):
        c = np.concatenate([rope_cos, rope_cos], axis=-1)[None, None]
        s = np.concatenate([rope_sin, rope_sin], axis=-1)[None, None]
        t1, t2 = np.split(t, 2, axis=-1)
        rot = np.concatenate([-t2, t1], axis=-1)
        return t * c + rot * s

    q = (x @ Wq).reshape(bsz, seq_len, 2 * H, D)
    q1 = np.transpose(q[:, :, :H], (0, 2, 1, 3))
    q2 = np.transpose(q[:, :, H:], (0, 2, 1, 3))
    k = (x @ Wk).reshape(bsz, seq_len, 2 * KV, D)
    k1 = np.transpose(k[:, :, :KV], (0, 2, 1, 3))
    k2 = np.transpose(k[:, :, KV:], (0, 2, 1, 3))
    v = np.transpose((x @ Wv).reshape(bsz, seq_len, KV, D), (0, 2, 1, 3))
    q1, q2, k1, k2 = rope(q1), rope(q2), rope(k1), rope(k2)
    gr = H // KV
    k1 = np.repeat(k1, gr, axis=1)
    k2 = np.repeat(k2, gr, axis=1)
    v = np.repeat(v, gr, axis=1)
    scale = 1.0 / math.sqrt(D)

    def smax(a):
        a = a - a.max(axis=-1, keepdims=True)
        e = np.exp(a)
        return e / e.sum(axis=-1, keepdims=True)

    a1 = smax(np.einsum("bhqd,bhkd->bhqk", q1, k1) * scale + attention_mask)
    a2 = smax(np.einsum("bhqd,bhkd->bhqk", q2, k2) * scale + attention_mask)
    lam = _compute_lam(lambda_q1, lambda_k1, lambda_q2, lambda_k2)
    attn = a1 - lam * a2
    out = np.einsum("bhqk,bhkd->bhqd", attn, v)
    inv = 1.0 / np.sqrt(np.mean(out * out, axis=-1, keepdims=True) + EPS)
    out = out * inv * subln_w
    out = np.transpose(out, (0, 2, 1, 3)).reshape(bsz, seq_len, HS)
    return (out @ Wo).astype(np.float32)


LAST_RESULT = None


def kernel(x, rope_cos, rope_sin, attention_mask, Wq, Wk, Wv, Wo,
           lambda_q1, lambda_k1, lambda_q2, lambda_k2, subln_w):
    global LAST_RESULT
    x = np.asarray(x, np.float32)
    kk, qq = np.arange(S)[:, None], np.arange(S)[None, :]
    causal = np.where(qq <= kk, 0.0, NEG).astype(np.float32)[None, None]
    am = np.asarray(attention_mask, np.float32)
    if am.shape != (1, 1, S, S) or not np.array_equal(am, causal):
        return _numpy_reference(x, rope_cos, rope_sin, am, Wq, Wk, Wv, Wo,
                                lambda_q1, lambda_k1, lambda_q2, lambda_k2,
                                subln_w)

    lam = _compute_lam(lambda_q1, lambda_k1, lambda_q2, lambda_k2)
    nc = get_program(lam)
    in_maps = _host_inputs(x, np.asarray(rope_cos, np.float32),
                           np.asarray(rope_sin, np.float32),
                           np.asarray(Wq, np.float32), np.asarray(Wk, np.float32),
                           np.asarray(Wv, np.float32), np.asarray(Wo, np.float32),
                           np.asarray(subln_w, np.float32), lam)
    res = bass_utils.run_bass_kernel_spmd(nc, in_maps, core_ids=list(range(8)))
    LAST_RESULT = res
    y = np.zeros((B, S, HS), np.float32)
    for c in range(8):
        y[c // 4] += res.results[c]["out_pt"].T.astype(np.float32)
    return y


# revision 10
# speedup vs baseline: 1.2356x; 1.0341x over previous
"""Differential attention (B=2, S=2048, HS=1024, H=16, KV=4, D=64) on 8 trn2 cores.

Sharding: core c = (b, g) with b = c // 4 (data parallel on batch) and
g = c % 4 (tensor parallel over the 4 KV head groups; each core owns the
4 query heads of its group).  Each core computes its 4 heads' normed
attention output and a row-parallel partial of the output projection
(out_pt = (O_heads @ Wo_rows)^T); the host sums the 4 partials per batch.

All matmul operands are bf16 (PSUM accumulation stays fp32), which the
cost model runs at 1 cycle/row with no small-tile penalty and which
halves DMA traffic and enables the DVE 2x/4x perf modes on the
element-wise tail.  RoPE: instead of a second full-contraction projection
with rotated weights, rot(Q)^T is produced by a cheap 128-contraction
matmul against a signed permutation matrix (rot(q)[d<32] = -q[d+32],
rot(q)[d>=32] = +q[d-32], applied per 64-block): q_sb = copy(psq) on
Pool, psqr = perm^T @ q_sb on PE, then rope is q_sb*cos + psqr*sin on
DVE.  V^T is computed directly per 128-wide k tile (lhsT = x^T chunk,
rhs = Wv chunk) so no PE transposes or extra copies are needed.

attention(qt), per head: flash-style causal attention over k tiles,
S^T[k,q] strips via two 64-contraction matmuls, P = exp(S/8) on ACT
(no row-max: scores are O(5) so exp is safe; S/exp emitted STAGE k-tiles
ahead of the U matmuls), causal diagonal wedge zeroed by affine_select
(split across DVE and Pool), U^T[128,q] += [V|ones].T @ P — the ones
block replicates the softmax denominator onto partitions 64..127, so the
epilogue is a lane-aligned reciprocal + one SBUF->SBUF partition shift +
O = U1/r1 - lam*U2/r2 (lam folded into V2, subtract on gpsimd).

rms(qt): O^2 row-sums via four ones-column matmuls into partitions
{0,32,64,96} of one PSUM tile, a single strided-partition Sqrt on ACT +
reciprocal on DVE, gpsimd partition-broadcast, subln_w folded into Wo.

wo(qt): partial^T = Wo_rows.T @ O_norm^T -> DRAM (bf16, host upcasts).

The emission is software-pipelined by q-tile — proj(qt) -> rms(qt-1) ->
attention(qt) -> wo(qt-1) — so the in-order engine queues never
head-of-line block ready work behind the RMS/output latency chains.
"""

import math
import sys

import numpy as np

try:
    import concourse.bass as bass  # noqa: F401
except ImportError:
    sys.path.insert(0, "/opt/trn_rl_repo")

import concourse.bass as bass
import concourse.tile as tile
from concourse import bacc, mybir
from concourse import bass_utils

f32 = mybir.dt.float32
bf16 = mybir.dt.bfloat16
AF = mybir.ActivationFunctionType
ALU = mybir.AluOpType

B, S, HS = 2, 2048, 1024
H, KV, D = 16, 4, 64
NHL = 4            # query heads per core
NQT = 4            # q tiles of 512
QTW = 512
NKT = 16           # k tiles of 128
NHS = 8            # hs tiles of 128
NEG = -1e9
EPS = 1e-5

_prog_cache = {}


def _build_program(lam: float):
    nc = bacc.Bacc("TRN2", target_bir_lowering=False, debug=False,
                   enable_asserts=False, num_devices=8)

    xt = nc.dram_tensor("xt", [HS, S], bf16, kind="ExternalInput").ap()
    wq = nc.dram_tensor("wq", [HS, 512], bf16, kind="ExternalInput").ap()
    wk = nc.dram_tensor("wk", [HS, 128], bf16, kind="ExternalInput").ap()
    wv = nc.dram_tensor("wv", [HS, 64], bf16, kind="ExternalInput").ap()
    wo = nc.dram_tensor("wo", [256, HS], bf16, kind="ExternalInput").ap()
    perm = nc.dram_tensor("perm", [128, 128], bf16, kind="ExternalInput").ap()
    cos_t = nc.dram_tensor("cos_t", [128, S], bf16, kind="ExternalInput").ap()
    sin_t = nc.dram_tensor("sin_t", [128, S], bf16, kind="ExternalInput").ap()
    ones = nc.dram_tensor("ones", [128, 64], bf16, kind="ExternalInput").ap()
    trimask = nc.dram_tensor("trimask", [128, 128], bf16,
                             kind="ExternalInput").ap()
    out_pt = nc.dram_tensor("out_pt", [HS, S], bf16, kind="ExternalOutput").ap()

    with tile.TileContext(nc) as tc:
        with tc.tile_pool(name="persist", bufs=1) as pp, \
             tc.tile_pool(name="loc", bufs=2) as loc, \
             tc.tile_pool(name="pwk", bufs=2) as pwk, \
             tc.tile_pool(name="patt", bufs=5) as pa, \
             tc.tile_pool(name="ep", bufs=2) as pe, \
             tc.tile_pool(name="rmsp", bufs=2) as prm, \
             tc.psum_pool(name="ps", bufs=2) as ps_:

            _dma_engines = [nc.sync, nc.scalar]
            _dma_i = [0]

            def dma_rr(dst, src):
                eng = _dma_engines[_dma_i[0] % 2]
                _dma_i[0] += 1
                eng.dma_start(dst, src)

            # load order = first-use order: wq feeds the very first matmuls
            wq_sb, wk_sb, wv_sb = [], [], []
            for hs in range(NHS):
                t_ = pp.tile([128, 512], bf16, name=f"wq{hs}", tag=f"wq{hs}")
                dma_rr(t_[:], wq[hs * 128:(hs + 1) * 128, :])
                wq_sb.append(t_)
            perm_sb = pp.tile([128, 128], bf16, name="perm", tag="perm")
            dma_rr(perm_sb[:], perm[:])
            for hs in range(NHS):
                t_ = pp.tile([128, 128], bf16, name=f"wk{hs}", tag=f"wk{hs}")
                dma_rr(t_[:], wk[hs * 128:(hs + 1) * 128, :])
                wk_sb.append(t_)
                t_ = pp.tile([128, 64], bf16, name=f"wv{hs}", tag=f"wv{hs}")
                dma_rr(t_[:], wv[hs * 128:(hs + 1) * 128, :])
                wv_sb.append(t_)
            cos_sb = pp.tile([128, S], bf16, name="cos", tag="cos")
            dma_rr(cos_sb[:], cos_t[:])
            sin_sb = pp.tile([128, S], bf16, name="sin", tag="sin")
            dma_rr(sin_sb[:], sin_t[:])
            ones_sb = pp.tile([128, 64], bf16, name="ones", tag="ones")
            dma_rr(ones_sb[:], ones[:])
            tri_sb = pp.tile([128, 128], bf16, name="tri", tag="tri")
            dma_rr(tri_sb[:], trimask[:])
            wo_sb = []
            for t in range(2):
                w = pp.tile([128, HS], bf16, name=f"wo{t}", tag=f"wo{t}")
                dma_rr(w[:], wo[t * 128:(t + 1) * 128, :])
                wo_sb.append(w)
            k_sb = pp.tile([128, S], bf16, name="k", tag="k")
            va = [pp.tile([128, 128], bf16, name=f"va{kt}", tag=f"va{kt}")
                  for kt in range(NKT)]
            vb = [pp.tile([128, 128], bf16, name=f"vb{kt}", tag=f"vb{kt}")
                  for kt in range(NKT)]
            for kt in range(NKT):
                nc.vector.tensor_copy(va[kt][:, 64:128], ones_sb[:])
                nc.vector.tensor_copy(vb[kt][:, 64:128], ones_sb[:])
            eps_sb = pp.tile([128, 1], f32, name="eps", tag="eps")
            nc.vector.memset(eps_sb[:], EPS)
            # preload the act-func set that holds BOTH Exp and Ln so the
            # table-load pass never has to switch sets mid-stream
            from concourse.hw_specs import get_activation_tables
            _tables = list(get_activation_tables(nc.m.arch).items())
            _set_id = next(i for i, (_, fs) in enumerate(_tables)
                           if AF.Exp in fs and AF.Ln in fs)
            _ld = mybir.InstLoadActFuncSet(
                name=nc.get_next_instruction_name(),
                act_func_set_id=_set_id, ins=[], outs=[])
            nc.scalar.add_instruction(_ld)

            def rope_block(ps, dst, qlo, qhi, dst_sb=None):
                # dst = q*cos + rot(q)*sin; rot via perm matmul on PE
                q_sb = dst_sb
                if q_sb is None:
                    q_sb = pwk.tile([128, QTW], bf16, name="qsb", tag="qsb")
                nc.gpsimd.tensor_copy(q_sb[:], ps[:])
                psr = ps_.tile([128, QTW], f32, name="psr", tag="aux")
                nc.tensor.matmul(psr[:], perm_sb[:], q_sb[:],
                                 start=True, stop=True)
                qc = pwk.tile([128, QTW], bf16, name="qc", tag="qc")
                nc.vector.tensor_mul(qc[:], q_sb[:], cos_sb[:, qlo:qhi])
                qs = pwk.tile([128, QTW], bf16, name="qs", tag="qs")
                nc.vector.tensor_mul(qs[:], psr[:], sin_sb[:, qlo:qhi])
                nc.vector.tensor_add(dst, qc[:], qs[:])

            def emit_xt_loads(qt, state):
                qlo, qhi = qt * QTW, (qt + 1) * QTW
                xt_sb = []
                for hs in range(NHS):
                    t_ = pwk.tile([128, QTW], bf16, name=f"xt{hs}", tag=f"xt{hs}")
                    eng = nc.scalar if hs % 2 == 0 else nc.sync
                    eng.dma_start(t_[:], xt[hs * 128:(hs + 1) * 128, qlo:qhi])
                    xt_sb.append(t_)
                state[(qt, "xt")] = xt_sb

            def emit_proj_piece(qt, piece, state):
                qlo, qhi = qt * QTW, (qt + 1) * QTW
                xt_sb = state[(qt, "xt")]
                if piece == 0:
                    psk = ps_.tile([128, QTW], f32, name="psk", tag="aux")
                    for hs in range(NHS):
                        nc.tensor.matmul(psk[:], wk_sb[hs][:], xt_sb[hs][:],
                                         start=(hs == 0), stop=(hs == NHS - 1))
                    rope_block(psk, k_sb[:, qlo:qhi], qlo, qhi)
                elif piece <= 4:
                    j = piece - 1
                    if j == 0:
                        state[qt] = [loc.tile([128, QTW], bf16, name=f"q{jj}",
                                              tag=f"q{jj}")
                                     for jj in range(NHL)]
                    qloc = state[qt]
                    psq = ps_.tile([128, QTW], f32, name="psq", tag="aux")
                    for hs in range(NHS):
                        nc.tensor.matmul(
                            psq[:], wq_sb[hs][:, j * 128:(j + 1) * 128],
                            xt_sb[hs][:], start=(hs == 0), stop=(hs == NHS - 1))
                    rope_block(psq, qloc[j][:], qlo, qhi)
                else:
                    for kk in range(4):
                        kt = 4 * qt + kk
                        psvt = ps_.tile([128, 64], f32, name="psvt", tag="aux")
                        for hs in range(NHS):
                            nc.tensor.matmul(
                                psvt[:], xt_sb[hs][:, kk * 128:(kk + 1) * 128],
                                wv_sb[hs][:], start=(hs == 0),
                                stop=(hs == NHS - 1))
                        nc.vector.tensor_copy(va[kt][:, 0:64], psvt[:])
                        nc.vector.tensor_scalar_mul(vb[kt][:, 0:64], psvt[:],
                                                    lam)

            def emit_att_head(qt, j, state):
                qloc = state[qt]
                if j == 0:
                    state[(qt, "op")] = [loc.tile([128, QTW], bf16,
                                                  name=f"op{t}", tag=f"op{t}")
                                         for t in range(2)]
                    state[(qt, "on")] = [loc.tile([128, QTW], bf16,
                                                  name=f"on{t}", tag=f"on{t}")
                                         for t in range(2)]
                opair = state[(qt, "op")]
                half, pt = (j % 2) * 64, j // 2
                last_kt = 4 * qt + 3
                psu = ps_.tile([128, 2 * QTW], f32, name="psu", tag="psU",
                               bufs=1)
                p12s = {}

                def emit_s_exp(kt):
                    jd = kt - 4 * qt
                    q0 = 128 * jd if jd >= 0 else 0
                    pss = ps_.tile([128, 2 * QTW], f32, name="pss", tag="psS")
                    nc.tensor.matmul(
                        pss[:, q0:QTW],
                        k_sb[0:64, kt * 128:(kt + 1) * 128],
                        qloc[j][0:64, q0:QTW],
                        start=True, stop=True, skip_group_check=True)
                    nc.tensor.matmul(
                        pss[:, QTW + q0:2 * QTW],
                        k_sb[64:128, kt * 128:(kt + 1) * 128],
                        qloc[j][64:128, q0:QTW],
                        start=True, stop=True, skip_group_check=True)
                    p12 = pa.tile([128, 2 * QTW], bf16, name="p12", tag="p12")
                    nc.scalar.activation(p12[:, q0:2 * QTW], pss[:, q0:2 * QTW],
                                         AF.Exp, scale=0.125)
                    if jd >= 0:
                        wap = p12[:].rearrange("p (b q) -> p b q",
                                               b=2)[:, :, q0:q0 + 128]
                        msk = tri_sb[:].unsqueeze(1).broadcast_to(
                            [128, 2, 128])
                        nc.vector.tensor_mul(wap, wap, msk)
                    p12s[kt] = p12

                STAGE = 5
                for kt in range(min(STAGE, last_kt + 1)):
                    emit_s_exp(kt)
                for kt in range(last_kt + 1):
                    if kt + STAGE <= last_kt:
                        emit_s_exp(kt + STAGE)
                    jd = kt - 4 * qt
                    q0 = 128 * jd if jd >= 0 else 0
                    p12 = p12s.pop(kt)
                    nc.tensor.matmul(
                        psu[:, q0:QTW], va[kt][:], p12[:, q0:QTW],
                        start=(kt == 0), stop=(kt == last_kt),
                        skip_group_check=True)
                    nc.tensor.matmul(
                        psu[:, QTW + q0:2 * QTW], vb[kt][:],
                        p12[:, QTW + q0:2 * QTW],
                        start=(kt == 0), stop=(kt == last_kt),
                        skip_group_check=True)
                # epilogue: O^T = U1/r1 - lam*U2/r2  (no PE ops here)
                wri = pe.tile([128, 2 * QTW], f32, name="wri", tag="wri")
                nc.vector.reciprocal(wri[64:128, :], psu[64:128, :])
                nc.sync.dma_start(wri[0:64, :], wri[64:128, :])
                t1 = pe.tile([64, QTW], bf16, name="t1", tag="t1")
                nc.vector.tensor_mul(t1[:], psu[0:64, 0:QTW], wri[0:64, 0:QTW])
                t2 = pe.tile([64, QTW], bf16, name="t2", tag="t2")
                nc.vector.tensor_mul(t2[:], psu[0:64, QTW:2 * QTW],
                                     wri[0:64, QTW:2 * QTW])
                if j % 2 == 0:
                    nc.gpsimd.tensor_sub(opair[pt][0:64, :], t1[:], t2[:])
                else:
                    otmp = pe.tile([64, QTW], bf16, name="otmp", tag="otmp")
                    nc.gpsimd.tensor_sub(otmp[:], t1[:], t2[:])
                    nc.sync.dma_start(opair[pt][64:128, :], otmp[:])

            def emit_rms(qt, state):
                opair = state[(qt, "op")]
                onq = state[(qt, "on")]
                sq4 = ps_.tile([128, QTW], f32, name="sq4", tag="aux")
                osqs = []
                for pt in range(2):
                    osq = prm.tile([128, QTW], bf16, name=f"osq{pt}",
                                   tag=f"osq{pt}")
                    nc.vector.tensor_mul(osq[:], opair[pt][:], opair[pt][:])
                    osqs.append(osq)
                for j in range(NHL):
                    half, pt = (j % 2) * 64, j // 2
                    nc.tensor.matmul(sq4[32 * j:32 * j + 1, :],
                                     ones_sb[half:half + 64, 0:1],
                                     osqs[pt][half:half + 64, :],
                                     start=True, stop=True,
                                     skip_group_check=True,
                                     tile_position=(half, 32 * j))
                # rms factor = exp(-0.5*ln(ssq/64+eps)) on 4 strided
                # partitions; Ln/Exp share the softmax act table (no reloads)
                sqr = prm.tile([128, QTW], f32, name="sqr", tag="sqr")
                nc.scalar.activation(sqr[0:128:32, :], sq4[0:128:32, :],
                                     AF.Ln, scale=1.0 / 64.0,
                                     bias=eps_sb[0:128:32, 0:1])
                rmq = prm.tile([128, QTW], bf16, name="rmq", tag="rmq")
                nc.scalar.activation(rmq[0:128:32, :], sqr[0:128:32, :],
                                     AF.Exp, scale=-0.5)
                for pt in range(2):
                    rsb = prm.tile([128, QTW], bf16, name="rsb", tag="rsb",
                                   bufs=1)
                    nc.gpsimd.partition_broadcast(
                        rsb[0:64, :], rmq[64 * pt:64 * pt + 1, :])
                    nc.gpsimd.partition_broadcast(
                        rsb[64:128, :], rmq[64 * pt + 32:64 * pt + 33, :])
                    nc.vector.tensor_mul(onq[pt][:], opair[pt][:], rsb[:])

            def emit_wo(qt, state, half):
                qlo, qhi = qt * QTW, (qt + 1) * QTW
                onq = state[(qt, "on")]
                for oc in range(4 * half, 4 * half + 4):
                    psw = ps_.tile([128, QTW], f32, name="psw", tag="aux")
                    nc.tensor.matmul(psw[:], wo_sb[0][:, oc * 128:(oc + 1) * 128],
                                     onq[0][:], start=True, stop=False)
                    nc.tensor.matmul(psw[:], wo_sb[1][:, oc * 128:(oc + 1) * 128],
                                     onq[1][:], start=False, stop=True)
                    ow = prm.tile([128, QTW], bf16, name="ow", tag="ow")
                    if oc % 2 == 0:
                        nc.vector.tensor_copy(ow[:], psw[:])
                    else:
                        nc.gpsimd.tensor_copy(ow[:], psw[:])
                    nc.sync.dma_start(out_pt[oc * 128:(oc + 1) * 128, qlo:qhi],
                                      ow[:])

            state = {}
            emit_xt_loads(0, state)
            for piece in range(6):
                emit_proj_piece(0, piece, state)
            for qt in range(NQT):
                if qt < NQT - 1:
                    emit_xt_loads(qt + 1, state)
                for j in range(NHL):
                    emit_att_head(qt, j, state)
                    if qt < NQT - 1:
                        for piece in ((0, 1), (2,), (3,), (4, 5))[j]:
                            emit_proj_piece(qt + 1, piece, state)
                    if j == 1 and qt > 0:
                        emit_rms(qt - 1, state)
                    if j == 2 and qt > 0:
                        emit_wo(qt - 1, state, half=0)
                    if j == 3 and qt > 0:
                        emit_wo(qt - 1, state, half=1)
            emit_rms(NQT - 1, state)
            emit_wo(NQT - 1, state, half=0)
            emit_wo(NQT - 1, state, half=1)
    nc.compile()
    return nc


def get_program(lam: float):
    key = round(float(lam), 9)
    if key not in _prog_cache:
        _prog_cache[key] = _build_program(float(lam))
    return _prog_cache[key]


def _bf16():
    import ml_dtypes
    return ml_dtypes.bfloat16


def _perm_mat():
    # psr = perm.T @ q : psr[p] = -q[p+32] for p%64<32, +q[p-32] for p%64>=32
    p = np.zeros((128, 128), np.float32)
    for o in range(128):
        if o % 64 < 32:
            p[o + 32, o] = -1.0
        else:
            p[o - 32, o] = 1.0
    return p


def _host_inputs(x, rope_cos, rope_sin, Wq, Wk, Wv, Wo, subln_w, lam):
    bf = _bf16()
    cos_t = np.ascontiguousarray(np.tile(rope_cos.T, (4, 1))).astype(bf)
    sin_t = np.ascontiguousarray(np.tile(rope_sin.T, (4, 1))).astype(bf)
    perm = _perm_mat().astype(bf)
    ones = np.ones((128, 64), np.float32).astype(bf)
    tri = np.triu(np.ones((128, 128), np.float32)).astype(bf)
    sub4 = np.tile(subln_w.astype(np.float32), 4)[:, None]

    in_maps = []
    for c in range(8):
        b, g = c // 4, c % 4
        xtc = np.ascontiguousarray(x[b].T).astype(bf)
        cols = []
        for j in range(NHL):
            h = 4 * g + j
            cols.append(Wq[:, h * 64:(h + 1) * 64])
            cols.append(Wq[:, (H + h) * 64:(H + h + 1) * 64])
        wq_c = np.ascontiguousarray(np.concatenate(cols, axis=1)).astype(bf)
        wk_c = np.ascontiguousarray(np.concatenate(
            [Wk[:, g * 64:(g + 1) * 64], Wk[:, (KV + g) * 64:(KV + g + 1) * 64]],
            axis=1)).astype(bf)
        wv_c = np.ascontiguousarray(Wv[:, g * 64:(g + 1) * 64]).astype(bf)
        wo_c = np.ascontiguousarray(
            Wo[g * 256:(g + 1) * 256, :] * sub4).astype(bf)
        in_maps.append({
            "xt": xtc, "wq": wq_c, "wk": wk_c, "wv": wv_c, "wo": wo_c,
            "perm": perm, "cos_t": cos_t, "sin_t": sin_t, "ones": ones,
            "trimask": tri,
        })
    return in_maps


def _compute_lam(lambda_q1, lambda_k1, lambda_q2, lambda_k2):
    li = 0.8 - 0.6 * math.exp(-0.3)
    l1 = np.exp(np.dot(lambda_q1.astype(np.float32), lambda_k1.astype(np.float32)))
    l2 = np.exp(np.dot(lambda_q2.astype(np.float32), lambda_k2.astype(np.float32)))
    return float(l1 - l2 + li)


def _numpy_reference(x, rope_cos, rope_sin, attention_mask, Wq, Wk, Wv, Wo,
                     lambda_q1, lambda_k1, lambda_q2, lambda_k2, subln_w):
    """Pure-numpy fallback, only used if the mask is not the expected causal one."""
    bsz, seq_len, _ = x.shape

    def rope(t):
        c = np.concatenate([rope_cos, rope_cos], axis=-1)[None, None]
        s = np.concatenate([rope_sin, rope_sin], axis=-1)[None, None]
        t1, t2 = np.split(t, 2, axis=-1)
        rot = np.concatenate([-t2, t1], axis=-1)
        return t * c + rot * s

    q = (x @ Wq).reshape(bsz, seq_len, 2 * H, D)
    q1 = np.transpose(q[:, :, :H], (0, 2, 1, 3))
    q2 = np.transpose(q[:, :, H:], (0, 2, 1, 3))
    k = (x @ Wk).reshape(bsz, seq_len, 2 * KV, D)
    k1 = np.transpose(k[:, :, :KV], (0, 2, 1, 3))
    k2 = np.transpose(k[:, :, KV:], (0, 2, 1, 3))
    v = np.transpose((x @ Wv).reshape(bsz, seq_len, KV, D), (0, 2, 1, 3))
    q1, q2, k1, k2 = rope(q1), rope(q2), rope(k1), rope(k2)
    gr = H // KV
    k1 = np.repeat(k1, gr, axis=1)
    k2 = np.repeat(k2, gr, axis=1)
    v = np.repeat(v, gr, axis=1)
    scale = 1.0 / math.sqrt(D)

    def smax(a):
        a = a - a.max(axis=-1, keepdims=True)
        e = np.exp(a)
        return e / e.sum(axis=-1, keepdims=True)

    a1 = smax(np.einsum("bhqd,bhkd->bhqk", q1, k1) * scale + attention_mask)
    a2 = smax(np.einsum("bhqd,bhkd->bhqk", q2, k2) * scale + attention_mask)
    lam = _compute_lam(lambda_q1, lambda_k1, lambda_q2, lambda_k2)
    attn = a1 - lam * a2
    out = np.einsum("bhqk,bhkd->bhqd", attn, v)
    inv = 1.0 / np.sqrt(np.mean(out * out, axis=-1, keepdims=True) + EPS)
    out = out * inv * subln_w
    out = np.transpose(out, (0, 2, 1, 3)).reshape(bsz, seq_len, HS)
    return (out @ Wo).astype(np.float32)


LAST_RESULT = None


def kernel(x, rope_cos, rope_sin, attention_mask, Wq, Wk, Wv, Wo,
           lambda_q1, lambda_k1, lambda_q2, lambda_k2, subln_w):
    global LAST_RESULT
    x = np.asarray(x, np.float32)
    kk, qq = np.arange(S)[:, None], np.arange(S)[None, :]
    causal = np.where(qq <= kk, 0.0, NEG).astype(np.float32)[None, None]
    am = np.asarray(attention_mask, np.float32)
    if am.shape != (1, 1, S, S) or not np.array_equal(am, causal):
        return _numpy_reference(x, rope_cos, rope_sin, am, Wq, Wk, Wv, Wo,
                                lambda_q1, lambda_k1, lambda_q2, lambda_k2,
                                subln_w)

    lam = _compute_lam(lambda_q1, lambda_k1, lambda_q2, lambda_k2)
    nc = get_program(lam)
    in_maps = _host_inputs(x, np.asarray(rope_cos, np.float32),
                           np.asarray(rope_sin, np.float32),
                           np.asarray(Wq, np.float32), np.asarray(Wk, np.float32),
                           np.asarray(Wv, np.float32), np.asarray(Wo, np.float32),
                           np.asarray(subln_w, np.float32), lam)
    res = bass_utils.run_bass_kernel_spmd(nc, in_maps, core_ids=list(range(8)))
    LAST_RESULT = res
    y = np.zeros((B, S, HS), np.float32)
    for c in range(8):
        y[c // 4] += res.results[c]["out_pt"].T.astype(np.float32)
    return y
